# revision 33
# baseline (speedup 1.0000x reference)
"""Trainium2 Bass kernel for nn_BaseGCNModel_addSE (gnn_message_passing).

SPMD over 8 NeuronCores, data laid out so the SE gate commutes with the
sparse aggregation:

    agg = A @ (x * (1+gate)) = (A @ x) * (1+gate)

since the gate is constant along the contracted node axis. The kernel
gathers messages directly from the host-marshalled node-major table
xt [N, B*F] (fp16, 1KB rows), segment-sums them on the PE via streamed
fp16 one-hot blocks (edge weights folded in), and applies the gate by
scaling Wg per batch. Every core owns 16 of the 128 dst-node tiles; BN1
is node-local; pooled partials are combined with AllGather + local max
(cheaper than AllReduce in the fabric); the FC head runs replicated in
feature-major layout (no transposes, per-partition bias/scale on ACT).

Mid-pipeline runs in fp16: agg is evacuated to fp16, transposed in
128x128 pairs on the PE (fp16 identity, 1 cyc/row), and the gate-scaled
Wg matmuls run fp16 (4x cheaper than the fp32 path).
"""

import os
import sys

for _p in ("/opt/trn_rl_repo", "/root/.axon_site/_ro/trn_rl_repo"):
    if _p not in sys.path:
        sys.path.insert(0, _p)

import numpy as np

import concourse.bass as bass
import concourse.bacc as bacc
import concourse.mybir as mybir
import concourse.tile as tile
from concourse.bass_utils import run_bass_kernel_spmd
from concourse.masks import make_identity

f16 = np.float16
F32 = mybir.dt.float32
F16 = mybir.dt.float16
I16 = mybir.dt.int16
AF = mybir.ActivationFunctionType
ALU = mybir.AluOpType
AX = mybir.AxisListType

B, N, F, E, H = 8, 16384, 64, 262144, 128
SE_D = 32
FC1, FC2, OUT = 256, 128, 4
BN_EPS = 1e-3
NCORES = 8
NTILE = 128            # global 128-node dst tiles
TPC = NTILE // NCORES  # dst tiles per core (16)
BF = B * F             # 512, xt row width
MAX_GATHER = 512      # SWDGE ring limit: >1024 descriptors per gather crashes


def build_kernel(cpts, skip_collective: bool = False, phases: str = "GB"):
    """Build the SPMD program. cpts[i] = chunks (of 128 edges) for tile slot i
    (per-core tiles are sorted by descending edge count, so slot i's static
    size is the max of the i-th order statistic across cores)."""
    if isinstance(cpts, int):
        cpts = (cpts,) * TPC
    slots_i = [c * 128 for c in cpts]
    total_slots = sum(slots_i)
    offs_i = np.concatenate([[0], np.cumsum(slots_i)]).astype(int)
    nc = bacc.Bacc("TRN2", target_bir_lowering=False, debug=False,
                   num_devices=NCORES)

    # inputs (identical content on every core unless noted "per-core")
    xt = nc.dram_tensor("xt", [N, BF], F16, kind="ExternalInput")
    xs = nc.dram_tensor("xs", [N // NCORES, BF], F16, kind="ExternalInput")  # per-core x slice
    gidx = nc.dram_tensor("gidx", [128, total_slots // 16], I16, kind="ExternalInput")  # per-core
    # per-chunk (dloc, w) column pairs; S one-hot blocks are built on-chip
    dlw = nc.dram_tensor("dlw", [128, 2 * (total_slots // 128)], F32,
                         kind="ExternalInput")  # per-core
    iota16 = nc.dram_tensor("iota16", [128, 128], F16, kind="ExternalInput")
    bn1p = nc.dram_tensor("bn1p", [TPC, 128, 2], F32, kind="ExternalInput")             # per-core
    wpack = nc.dram_tensor("wpack", [128, 849], F32, kind="ExternalInput")
    bg4 = nc.dram_tensor("bg4", [1, BF], F16, kind="ExternalInput")  # bg tiled 4x

    out_t = nc.dram_tensor("out", [B, OUT], F32, kind="ExternalOutput")

    with tile.TileContext(nc) as tc:
        with (
            tc.tile_pool(name="const", bufs=1) as cpool,
            tc.tile_pool(name="sbuf", bufs=2) as pool,
            tc.tile_pool(name="psum", bufs=2, space="PSUM") as psum,
            tc.tile_pool(name="dram", bufs=1, space="DRAM") as dpool,
        ):
            # ---- constants / weights ----
            ident32 = cpool.tile([128, 128], F32)
            make_identity(nc, ident32[:])
            ident16 = cpool.tile([128, 128], F16)
            make_identity(nc, ident16[:])
            ones16 = cpool.tile([1, 128], F16)
            nc.vector.memset(ones16[:], 1.0)

            wpack_sb = cpool.tile([128, 849], F32)
            nc.sync.dma_start(out=wpack_sb[:], in_=wpack[:])
            bg4_sb = cpool.tile([1, BF], F16)
            nc.sync.dma_start(out=bg4_sb[:], in_=bg4[:])
            dlw_sb = cpool.tile([128, 2 * (total_slots // 128)], F32)
            nc.sync.dma_start(out=dlw_sb[:], in_=dlw[:])
            iota_sb = cpool.tile([128, 128], F16)
            nc.sync.dma_start(out=iota_sb[:], in_=iota16[:])
            wf1_sb = wpack_sb[:, 0:256]
            wf2a_sb = wpack_sb[:, 256:384]
            wf2b_sb = wpack_sb[:, 384:512]
            wg2_sb = wpack_sb[:, 512:640]
            wop2_sb = wpack_sb[0:SE_D, 640:768]
            w1_sb = wpack_sb[0:F, 768:800]
            w2_sb = wpack_sb[0:SE_D, 800:832]
            wo_sb = wpack_sb[:, 832:836]
            b1_sb = wpack_sb[0:SE_D, 836:837]
            b2_sb = wpack_sb[0:SE_D, 837:838]
            bop2_sb = wpack_sb[:, 838:839]
            bf1c_sb = wpack_sb[:, 839:841]
            bf2c_sb = wpack_sb[:, 841:842]
            g2_sb = wpack_sb[:, 842:844]
            be2_sb = wpack_sb[:, 844:846]
            g3_sb = wpack_sb[:, 846:847]
            be3_sb = wpack_sb[:, 847:848]
            boc_sb = wpack_sb[0:OUT, 848:849]

            # ---- phase G: SE gate (max-pool over nodes + tiny MLP) ----
            gates = None
            if "G" in phases:
                # sharded x-scan: each core reduces its N/8 slice, then
                # AllGather + local max of the [F, B] partials
                rows = N // NCORES
                half = rows // 2
                xs_sb = [cpool.tile([128, half // 128, BF], F16,
                                    tag=f"xs_sb{i}", name=f"xs_sb{i}")
                         for i in range(2)]
                reds = [cpool.tile([128, BF], F32, tag=f"reds{i}",
                                   name=f"reds{i}") for i in range(2)]
                for i in range(2):
                    nc.sync.dma_start(
                        out=xs_sb[i][:],
                        in_=xs[i * half:(i + 1) * half].rearrange(
                            "(p c) w -> p c w", p=128))
                    nc.vector.tensor_reduce(
                        out=reds[i][:],
                        in_=xs_sb[i][:].rearrange("p c w -> p w c"),
                        axis=AX.X, op=ALU.max,
                    )
                redpart = cpool.tile([128, BF], F32)
                nc.vector.tensor_tensor(out=redpart[:], in0=reds[0][:],
                                        in1=reds[1][:], op=ALU.max)
                pp = cpool.tile([F, B], F32)
                for b in range(B):
                    red_ps = psum.tile([F, 128], F32, space="PSUM",
                                       tag="ps_tr")
                    nc.tensor.transpose(
                        out=red_ps[:], in_=redpart[:, b * F:(b + 1) * F],
                        identity=ident32[:])
                    nc.vector.tensor_reduce(out=pp[:, b:b + 1], in_=red_ps[:],
                                            axis=AX.X, op=ALU.max)
                if skip_collective:
                    ppf = pp
                else:
                    r_in = dpool.tile([F, B], F32)
                    r_out = dpool.tile([NCORES, F, B], F32)
                    nc.gpsimd.dma_start(out=r_in[:], in_=pp[:])
                    nc.gpsimd.collective_compute(
                        "AllGather", ALU.bypass,
                        replica_groups=[list(range(NCORES))],
                        ins=[r_in.opt()], outs=[r_out.opt()],
                    )
                    ppg = cpool.tile([F, NCORES, B], F32)
                    nc.sync.dma_start(out=ppg[:],
                                      in_=r_out[:].rearrange("c p b -> p c b"))
                    ppf = cpool.tile([F, B], F32)
                    nc.vector.tensor_reduce(
                        out=ppf[:], in_=ppg[:].rearrange("p c b -> p b c"),
                        axis=AX.X, op=ALU.max)
                # gate MLP, all batches at once
                a1_ps = psum.tile([SE_D, B], F32, space="PSUM", tag="ps_b", bufs=1)
                nc.tensor.matmul(out=a1_ps[:], lhsT=w1_sb[:], rhs=ppf[:],
                                 start=True, stop=True)
                a1 = pool.tile([SE_D, B], F32, tag="a1")
                nc.scalar.activation(out=a1[:], in_=a1_ps[:], func=AF.Relu,
                                     bias=b1_sb[:])
                a2_ps = psum.tile([SE_D, B], F32, space="PSUM", tag="ps_b", bufs=1)
                nc.tensor.matmul(out=a2_ps[:], lhsT=w2_sb[:], rhs=a1[:],
                                 start=True, stop=True)
                a2 = pool.tile([SE_D, B], F32, tag="a2")
                nc.scalar.activation(out=a2[:], in_=a2_ps[:], func=AF.Relu,
                                     bias=b2_sb[:])
                g_ps = psum.tile([2 * F, B], F32, space="PSUM", tag="ps_b", bufs=1)
                nc.tensor.matmul(out=g_ps[:], lhsT=wop2_sb[:], rhs=a2[:],
                                 start=True, stop=True)
                gates = cpool.tile([2 * F, B], F32)
                nc.scalar.activation(out=gates[:], in_=g_ps[:],
                                     func=AF.Sigmoid, bias=bop2_sb[:])
                nc.vector.tensor_scalar_add(gates[:], gates[:], 1.0)

            # block-diagonal gate-scaled Wg pairs (fp16): for batch pair
            # (2j, 2j+1), rows 0:64 x cols 0:128 hold Wg*gate[2j] and rows
            # 64:128 x cols 128:256 hold Wg*gate[2j+1]. One h2 matmul then
            # computes both batches (contraction over the stacked f axis).
            wgebz = []
            for j in range(4):
                wz = cpool.tile([2 * F, 2 * H], F16, tag=f"wgebz_{j}",
                                name=f"wgebz_{j}")
                nc.vector.memset(wz[:], 0.0)
                if gates is not None:
                    nc.vector.tensor_scalar(
                        out=wz[0:F, 0:H], in0=wg2_sb[0:F, :],
                        scalar1=gates[0:F, 2 * j:2 * j + 1],
                        scalar2=None, op0=ALU.mult)
                    nc.vector.tensor_scalar(
                        out=wz[F:2 * F, H:2 * H], in0=wg2_sb[F:2 * F, :],
                        scalar1=gates[F:2 * F, 2 * j + 1:2 * j + 2],
                        scalar2=None, op0=ALU.mult)
                else:
                    nc.vector.tensor_copy(out=wz[0:F, 0:H],
                                          in_=wg2_sb[0:F, :])
                    nc.vector.tensor_copy(out=wz[F:2 * F, H:2 * H],
                                          in_=wg2_sb[F:2 * F, :])
                wgebz.append(wz)

            # ---- phase B: SpMM + Wg + BN1 + pool partial ----
            pooled = cpool.tile([H, B], F32)
            poolacc = [cpool.tile([128, BF], F16, tag=f"poolacc_{g}",
                                  name=f"poolacc_{g}")
                       for g in range(2)]

            for t in range(TPC) if "B" in phases else []:
                cpt_t = cpts[t]
                slots_t = slots_i[t]
                off_t = int(offs_i[t])
                gidx_sb = pool.tile([128, slots_t // 16], I16, tag="gidx_sb",
                                    bufs=3, name="gidx_sb")
                nc.sync.dma_start(out=gidx_sb[:],
                                  in_=gidx[:, off_t // 16:(off_t + slots_t) // 16])
                # build the one-hot S blocks on-chip: row e of chunk k gets
                # w at column dloc ((iota == dloc) * w), zeros elsewhere
                s_sb = pool.tile([128, slots_t], F16, tag="s_sb", bufs=4,
                                 name="s_sb")
                cbase = off_t // 128
                for k in range(cpt_t):
                    c = cbase + k
                    nc.vector.tensor_scalar(
                        out=s_sb[:, k * 128:(k + 1) * 128], in0=iota_sb[:],
                        scalar1=dlw_sb[:, 2 * c:2 * c + 1],
                        scalar2=dlw_sb[:, 2 * c + 1:2 * c + 2],
                        op0=ALU.is_equal, op1=ALU.mult)
                bn1_sb = pool.tile([128, 2], F32, tag="bn1_sb", bufs=3,
                                   name="bn1_sb")
                nc.sync.dma_start(out=bn1_sb[:], in_=bn1p[t])
                msg = pool.tile([128, cpt_t, BF], F16, tag="msg", bufs=5,
                                name="msg")
                for s0 in range(0, slots_t, MAX_GATHER):
                    n_i = min(MAX_GATHER, slots_t - s0)
                    nc.gpsimd.dma_gather(
                        out_ap=msg[:, s0 // 128:(s0 + n_i) // 128, :],
                        in_ap=xt[:],
                        idxs_ap=gidx_sb[:, s0 // 16:(s0 + n_i) // 16],
                        num_idxs=n_i, num_idxs_reg=n_i, elem_size=BF,
                    )
                agg_ps = psum.tile([128, BF], F32, space="PSUM", tag="ps_agg",
                                   bufs=3)
                for k in range(cpt_t):
                    nc.tensor.matmul(
                        out=agg_ps[:],
                        lhsT=s_sb[:, k * 128:(k + 1) * 128],
                        rhs=msg[:, k, :],
                        start=(k == 0), stop=(k == cpt_t - 1),
                    )
                agg32 = pool.tile([128, BF], F32, tag="agg32", bufs=3,
                                  name="agg32")
                nc.scalar.activation(out=agg32[:], in_=agg_ps[:], func=AF.Copy)
                # batch-pair transposes; aggT[:, j, :] holds batches 2j
                # (partitions 0:64) and 2j+1 (64:128), fp16-cast on evacuation
                aggT = pool.tile([128, 4, 128], F16, tag="aggT", bufs=6,
                                 name="aggT")
                for j in range(4):
                    tr_ps = psum.tile([128, 128], F32, space="PSUM",
                                      tag="ps_tr")
                    nc.tensor.transpose(
                        out=tr_ps[:], in_=agg32[:, j * 128:(j + 1) * 128],
                        identity=ident32[:])
                    eng = nc.vector if j % 2 == 0 else nc.scalar
                    if eng is nc.vector:
                        eng.tensor_copy(out=aggT[:, j, :], in_=tr_ps[:])
                    else:
                        eng.activation(out=aggT[:, j, :], in_=tr_ps[:],
                                       func=AF.Copy)
                # h2 = relu(agg_gated @ Wg + bg), 4 batches per PSUM bank via
                # block-diagonal pair weights
                sums = pool.tile([128, 2], F32, tag="sums", name="sums")
                sqs = pool.tile([128, 2], F32, tag="sqs", name="sqs")
                h2g = []
                for g in range(2):
                    h2_ps = psum.tile([128, BF], F32, space="PSUM",
                                      tag="ps_h2", bufs=2)
                    nc.tensor.matmul(out=h2_ps[:], lhsT=ones16[:],
                                     rhs=bg4_sb[:], start=True, stop=False)
                    for jj in range(2):
                        j = g * 2 + jj
                        nc.tensor.matmul(
                            out=h2_ps[:, jj * 2 * H:(jj + 1) * 2 * H],
                            lhsT=aggT[:, j, :],
                            rhs=wgebz[j][:],
                            start=False, stop=(jj == 1))
                    h2 = pool.tile([128, BF], F16, tag=f"h2_{g}", bufs=3,
                                   name=f"h2_{g}")
                    nc.scalar.activation(out=h2[:], in_=h2_ps[:], func=AF.Relu,
                                         accum_out=sums[:, g:g + 1])
                    sqscr = pool.tile([128, BF], F16, tag="sqscr", bufs=3,
                                      name="sqscr")
                    nc.vector.tensor_tensor(out=sqscr[:], in0=h2[:],
                                            in1=h2[:], op=ALU.mult)
                    nc.vector.tensor_reduce(out=sqs[:, g:g + 1],
                                            in_=sqscr[:], axis=AX.X,
                                            op=ALU.add)
                    h2g.append(h2)
                # BN1 per-node affine coefficients (DVE column math)
                rsumt = pool.tile([128, 1], F32, tag="rsumt", name="rsumt")
                nc.vector.tensor_reduce(out=rsumt[:], in_=sums[:], axis=AX.X,
                                        op=ALU.add)
                sqsumt = pool.tile([128, 1], F32, tag="sqsumt", name="sqsumt")
                nc.vector.tensor_reduce(out=sqsumt[:], in_=sqs[:], axis=AX.X,
                                        op=ALU.add)
                mean = pool.tile([128, 1], F32, tag="mean", name="mean")
                nc.vector.tensor_scalar_mul(mean[:], rsumt[:], 1.0 / (B * H))
                msqe = pool.tile([128, 1], F32, tag="msqe", name="msqe")
                nc.vector.tensor_scalar_mul(msqe[:], sqsumt[:], 1.0 / (B * H))
                nc.vector.tensor_scalar_add(msqe[:], msqe[:], BN_EPS)
                var = pool.tile([128, 1], F32, tag="var", name="var")
                nc.vector.tensor_tensor(out=var[:], in0=mean[:], in1=mean[:],
                                        op=ALU.mult)
                nc.vector.tensor_tensor(out=var[:], in0=msqe[:], in1=var[:],
                                        op=ALU.subtract)
                inv = pool.tile([128, 1], F32, tag="inv", name="inv")
                nc.vector.reciprocal(out=inv[:], in_=var[:])
                rstd = pool.tile([128, 1], F32, tag="rstd", name="rstd")
                nc.scalar.sqrt(out=rstd[:], in_=inv[:])
                aco = pool.tile([128, 1], F32, tag="aco", name="aco")
                nc.vector.tensor_tensor(out=aco[:], in0=rstd[:],
                                        in1=bn1_sb[:, 0:1], op=ALU.mult)
                bco = pool.tile([128, 1], F32, tag="bco", name="bco")
                nc.vector.tensor_tensor(out=bco[:], in0=mean[:], in1=aco[:],
                                        op=ALU.mult)
                nc.vector.tensor_tensor(out=bco[:], in0=bn1_sb[:, 1:2],
                                        in1=bco[:], op=ALU.subtract)
                # apply BN1 (ACT: out = aco*h2 + bco) and fold into pool max
                for g in range(2):
                    if t == 0:
                        nc.vector.tensor_scalar(
                            out=poolacc[g][:], in0=h2g[g][:],
                            scalar1=aco[:, 0:1], scalar2=bco[:, 0:1],
                            op0=ALU.mult, op1=ALU.add)
                    else:
                        h2n = pool.tile([128, BF], F16, tag="h2n", bufs=3,
                                        name="h2n")
                        nc.vector.tensor_scalar(
                            out=h2n[:], in0=h2g[g][:],
                            scalar1=aco[:, 0:1], scalar2=bco[:, 0:1],
                            op0=ALU.mult, op1=ALU.add)
                        nc.vector.tensor_tensor(out=poolacc[g][:],
                                                in0=poolacc[g][:], in1=h2n[:],
                                                op=ALU.max)

            # fold pooled partials: per batch, transpose + reduce over nodes
            pacc32 = [cpool.tile([128, BF], F32, tag=f"pacc32_{g}",
                                 name=f"pacc32_{g}") for g in range(2)]
            for g in range(2):
                nc.scalar.activation(out=pacc32[g][:], in_=poolacc[g][:],
                                     func=AF.Copy)
            for b in range(B):
                g, j = b // 4, b % 4
                hT_ps = psum.tile([128, 128], F32, space="PSUM", tag="ps_tr")
                nc.tensor.transpose(
                    out=hT_ps[:], in_=pacc32[g][:, j * H:(j + 1) * H],
                    identity=ident32[:])
                nc.vector.tensor_reduce(out=pooled[:, b:b + 1], in_=hT_ps[:],
                                        axis=AX.X, op=ALU.max)

            # ---- phase C: AllGather + local max + replicated head ----
            if skip_collective:
                pooledf = pooled
            else:
                p_in = dpool.tile([H, B], F32)
                p_out = dpool.tile([NCORES, H, B], F32)
                nc.gpsimd.dma_start(out=p_in[:], in_=pooled[:])
                nc.gpsimd.collective_compute(
                    "AllGather", ALU.bypass,
                    replica_groups=[list(range(NCORES))],
                    ins=[p_in.opt()], outs=[p_out.opt()],
                )
                pg = cpool.tile([H, NCORES, B], F32)
                nc.sync.dma_start(out=pg[:],
                                  in_=p_out[:].rearrange("c p b -> p c b"))
                pooledf = cpool.tile([H, B], F32)
                nc.vector.tensor_reduce(
                    out=pooledf[:], in_=pg[:].rearrange("p c b -> p b c"),
                    axis=AX.X, op=ALU.max)

            def bn_free8(z, nrows, gamma_col, beta_col, tag):
                """BN over the 8 free-dim entries of z [nrows, 8] -> new tile."""
                rs = pool.tile([nrows, 1], F32, tag=f"{tag}_rs")
                nc.vector.tensor_reduce(out=rs[:], in_=z[:], axis=AX.X,
                                        op=ALU.add)
                nc.vector.tensor_scalar_mul(rs[:], rs[:], 1.0 / B)
                sqt = pool.tile([nrows, B], F32, tag=f"{tag}_sqt")
                sq = pool.tile([nrows, 1], F32, tag=f"{tag}_sq")
                nc.scalar.activation(out=sqt[:], in_=z[:], func=AF.Square,
                                     accum_out=sq[:])
                nc.vector.tensor_scalar_mul(sq[:], sq[:], 1.0 / B)
                v = pool.tile([nrows, 1], F32, tag=f"{tag}_v")
                nc.vector.tensor_tensor(out=v[:], in0=rs[:], in1=rs[:],
                                        op=ALU.mult)
                nc.vector.tensor_tensor(out=v[:], in0=sq[:], in1=v[:],
                                        op=ALU.subtract)
                nc.vector.tensor_scalar_add(v[:], v[:], BN_EPS)
                iv = pool.tile([nrows, 1], F32, tag=f"{tag}_iv")
                nc.vector.reciprocal(out=iv[:], in_=v[:])
                rst = pool.tile([nrows, 1], F32, tag=f"{tag}_rst")
                nc.scalar.sqrt(out=rst[:], in_=iv[:])
                ac = pool.tile([nrows, 1], F32, tag=f"{tag}_ac")
                nc.vector.tensor_tensor(out=ac[:], in0=rst[:], in1=gamma_col,
                                        op=ALU.mult)
                bc = pool.tile([nrows, 1], F32, tag=f"{tag}_bc")
                nc.vector.tensor_tensor(out=bc[:], in0=rs[:], in1=ac[:],
                                        op=ALU.mult)
                nc.vector.tensor_tensor(out=bc[:], in0=beta_col, in1=bc[:],
                                        op=ALU.subtract)
                zn = pool.tile([nrows, B], F32, tag=f"{tag}_zn")
                nc.vector.tensor_scalar(out=zn[:], in0=z[:],
                                        scalar1=ac[:, 0:1], scalar2=bc[:, 0:1],
                                        op0=ALU.mult, op1=ALU.add)
                return zn

            # head in feature-major layout: features on partitions, batch on
            # free. Biases ride ACT's per-partition bias port; no transposes.
            z1n = []
            for j in range(2):
                z1_ps = psum.tile([H, B], F32, space="PSUM", tag="ps_b", bufs=1)
                nc.tensor.matmul(out=z1_ps[:],
                                 lhsT=wf1_sb[:, j * 128:(j + 1) * 128],
                                 rhs=pooledf[:], start=True, stop=True)
                z1T = pool.tile([H, B], F32, tag=f"z1T_{j}")
                nc.scalar.activation(out=z1T[:], in_=z1_ps[:], func=AF.Relu,
                                     bias=bf1c_sb[:, j:j + 1])
                z1n.append(bn_free8(z1T, 128, g2_sb[:, j:j + 1],
                                    be2_sb[:, j:j + 1], f"bn2_{j}"))
            z2_ps = psum.tile([FC2, B], F32, space="PSUM", tag="ps_b", bufs=1)
            nc.tensor.matmul(out=z2_ps[:], lhsT=wf2a_sb[:], rhs=z1n[0][:],
                             start=True, stop=False)
            nc.tensor.matmul(out=z2_ps[:], lhsT=wf2b_sb[:], rhs=z1n[1][:],
                             start=False, stop=True)
            z2T = pool.tile([FC2, B], F32, tag="z2T")
            nc.scalar.activation(out=z2T[:], in_=z2_ps[:], func=AF.Relu,
                                 bias=bf2c_sb[:])
            z2n = bn_free8(z2T, FC2, g3_sb[:, 0:1], be3_sb[:, 0:1], "bn3")
            # logits^T [OUT, B], add bias, transpose to [B, OUT], softmax
            lg_ps = psum.tile([OUT, B], F32, space="PSUM", tag="ps_b", bufs=1)
            nc.tensor.matmul(out=lg_ps[:], lhsT=wo_sb[:], rhs=z2n[:],
                             start=True, stop=True)
            lgT = pool.tile([OUT, B], F32, tag="lgT")
            nc.vector.tensor_scalar(out=lgT[:], in0=lg_ps[:],
                                    scalar1=boc_sb[:, 0:1], scalar2=None,
                                    op0=ALU.add)
            lgt_ps = psum.tile([B, OUT], F32, space="PSUM", tag="ps_b", bufs=1)
            nc.tensor.transpose(out=lgt_ps[:], in_=lgT[:],
                                identity=ident32[0:OUT, 0:OUT])
            lg = pool.tile([B, OUT], F32, tag="lg")
            nc.vector.tensor_copy(out=lg[:], in_=lgt_ps[:])
            mx = pool.tile([B, 1], F32, tag="mx")
            nc.vector.tensor_reduce(out=mx[:], in_=lg[:], axis=AX.X, op=ALU.max)
            ex = pool.tile([B, OUT], F32, tag="ex")
            nc.vector.tensor_scalar(out=ex[:], in0=lg[:], scalar1=mx[:, 0:1],
                                    scalar2=None, op0=ALU.subtract)
            nc.scalar.activation(out=ex[:], in_=ex[:], func=AF.Exp)
            ssum = pool.tile([B, 1], F32, tag="ssum")
            nc.vector.tensor_reduce(out=ssum[:], in_=ex[:], axis=AX.X,
                                    op=ALU.add)
            sinv = pool.tile([B, 1], F32, tag="sinv")
            nc.vector.reciprocal(out=sinv[:], in_=ssum[:])
            sm = pool.tile([B, OUT], F32, tag="sm")
            nc.vector.tensor_scalar(out=sm[:], in0=ex[:], scalar1=sinv[:, 0:1],
                                    scalar2=None, op0=ALU.mult)
            nc.sync.dma_start(out=out_t[:], in_=sm[:])
    nc.compile()
    return nc


def preprocess(x, src, dst, edge_w):
    """Host marshalling: node-major x table + sorted/padded edge tiles."""
    order = np.argsort(dst, kind="stable")
    ss = src[order].astype(np.int64)
    ds = dst[order].astype(np.int64)
    ws = edge_w[order].astype(np.float32)
    tile_id = ds // 128
    dloc = ds % 128
    counts = np.bincount(tile_id, minlength=NTILE)
    cpt = int(np.ceil(counts.max() / 128))
    slots = cpt * 128

    gidx_all = np.zeros((NTILE, slots), np.int16)
    sval = np.zeros((NTILE, slots), np.float32)
    sloc = np.zeros((NTILE, slots), np.int64)
    offs = np.concatenate([[0], np.cumsum(counts)])
    for t in range(NTILE):
        cnt = counts[t]
        seg = slice(offs[t], offs[t + 1])
        # order each tile's edges by ascending src so gather descriptors
        # read ascending HBM addresses (DRAM row-buffer locality); the
        # segment-sum is order-invariant since S follows the slot order
        o = np.argsort(ss[seg], kind="stable")
        gidx_all[t, :cnt] = ss[seg][o]
        sval[t, :cnt] = ws[seg][o]
        sloc[t, :cnt] = dloc[seg][o]

    # per-chunk (dloc, w) columns; padded slots get dloc=200 (unmatchable)
    dlocw = np.zeros((NTILE, 128, 2 * cpt), np.float32)
    for t in range(NTILE):
        dl = sloc[t].reshape(cpt, 128).T.astype(np.float32)
        wv = sval[t].reshape(cpt, 128).T
        dl[wv == 0.0] = 200.0
        dlocw[t, :, 0::2] = dl
        dlocw[t, :, 1::2] = wv
    dlocw = dlocw.astype(np.float32)

    # wrapped int16 index tables: [16, slots//16] replicated to 128 partitions
    gidx_w = np.zeros((NTILE, 128, slots // 16), np.int16)
    for t in range(NTILE):
        base = gidx_all[t].reshape(slots // 16, 16).T
        gidx_w[t] = np.tile(base, (8, 1))

    # per-core tile order (descending edge count) and per-slot chunk counts
    order_pc = np.zeros((NCORES, TPC), np.int64)
    for c in range(NCORES):
        tl = np.arange(c * TPC, (c + 1) * TPC)
        order_pc[c] = tl[np.argsort(-counts[tl], kind="stable")]
    cpts = tuple(
        int(np.ceil(max(counts[order_pc[c][i]] for c in range(NCORES)) / 128))
        for i in range(TPC)
    )
    xt = np.ascontiguousarray(
        np.asarray(x, np.float32).transpose(1, 0, 2).reshape(N, BF)
    ).astype(f16)
    return xt, gidx_w, dlocw, cpts, order_pc


def make_in_maps(inputs, xt, gidx_w, dlocw, cpts, order_pc):
    g1 = np.asarray(inputs["g1"], np.float32).reshape(NTILE, 128)
    beta1 = np.asarray(inputs["beta1"], np.float32).reshape(NTILE, 128)
    bn1 = np.stack([g1, beta1], axis=-1)  # [NTILE, 128, 2]

    f32 = lambda a: np.ascontiguousarray(np.asarray(a, np.float32))
    wp = np.zeros((128, 849), np.float32)
    wp[:, 0:256] = f32(inputs["Wf1"])
    wf2 = f32(inputs["Wf2"]).reshape(2, H, FC2)
    wp[:, 256:384] = wf2[0]
    wp[:, 384:512] = wf2[1]
    wp[:, 512:640] = np.tile(f32(inputs["Wg"]), (2, 1))
    wp[0:SE_D, 640:768] = np.concatenate([f32(inputs["Wop"])] * 2, axis=1)
    wp[0:F, 768:800] = f32(inputs["W1"])
    wp[0:SE_D, 800:832] = f32(inputs["W2"])
    wp[:, 832:836] = f32(inputs["Wo"])
    wp[0:SE_D, 836] = f32(inputs["b1"])
    wp[0:SE_D, 837] = f32(inputs["b2"])
    wp[:, 838] = np.tile(f32(inputs["bop"]), 2)
    wp[:, 839:841] = f32(inputs["bf1"]).reshape(2, H).T
    wp[:, 841] = f32(inputs["bf2"])
    wp[:, 842:844] = f32(inputs["g2"]).reshape(2, H).T
    wp[:, 844:846] = f32(inputs["beta2"]).reshape(2, H).T
    wp[:, 846] = f32(inputs["g3"])
    wp[:, 847] = f32(inputs["beta3"])
    wp[0:OUT, 848] = f32(inputs["bo"])
    shared = {
        "xt": xt,
        "iota16": np.tile(np.arange(128, dtype=np.float32), (128, 1)).astype(f16),
        "wpack": wp,
        "bg4": np.tile(f32(inputs["bg"]).reshape(1, H), (1, 4)).astype(f16),
    }
    in_maps = []
    for c in range(NCORES):
        order = order_pc[c]
        m = dict(shared)
        m["xs"] = np.ascontiguousarray(xt[c * (N // NCORES):(c + 1) * (N // NCORES)])
        m["gidx"] = np.ascontiguousarray(np.concatenate(
            [gidx_w[gt][:, :cpts[i] * 8] for i, gt in enumerate(order)], axis=1))
        m["dlw"] = np.ascontiguousarray(np.concatenate(
            [dlocw[gt][:, :2 * cpts[i]] for i, gt in enumerate(order)], axis=1))
        m["bn1p"] = np.ascontiguousarray(bn1[order])
        in_maps.append(m)
    return in_maps


_CACHE = {}
LAST_RESULT = None  # BassKernelResults of the most recent kernel() call


def kernel(**inputs):
    global LAST_RESULT
    xt, gidx_w, dlocw, cpts, order_pc = preprocess(
        np.asarray(inputs["x"]), np.asarray(inputs["src"]),
        np.asarray(inputs["dst"]), np.asarray(inputs["edge_w"]))
    if cpts not in _CACHE:
        _CACHE[cpts] = build_kernel(cpts)
    nc = _CACHE[cpts]
    in_maps = make_in_maps(inputs, xt, gidx_w, dlocw, cpts, order_pc)
    trace = os.environ.get("BASS_KERNEL_TRACE", "0") == "1"
    res = run_bass_kernel_spmd(nc, in_maps, list(range(NCORES)), trace=trace)
    LAST_RESULT = res
    return np.asarray(res.results[0]["out"], np.float32)


# revision 42
# speedup vs baseline: 1.0119x; 1.0119x over previous
"""Trainium2 Bass kernel for nn_BaseGCNModel_addSE (gnn_message_passing).

SPMD over 8 NeuronCores, data laid out so the SE gate commutes with the
sparse aggregation:

    agg = A @ (x * (1+gate)) = (A @ x) * (1+gate)

since the gate is constant along the contracted node axis. The kernel
gathers messages directly from the host-marshalled node-major table
xt [N, B*F] (fp16, 1KB rows), segment-sums them on the PE via streamed
fp16 one-hot blocks (edge weights folded in), and applies the gate by
scaling Wg per batch. Every core owns 16 of the 128 dst-node tiles; BN1
is node-local; pooled partials are combined with AllGather + local max
(cheaper than AllReduce in the fabric); the FC head runs replicated in
feature-major layout (no transposes, per-partition bias/scale on ACT).

Mid-pipeline runs in fp16: agg is evacuated to fp16, transposed in
128x128 pairs on the PE (fp16 identity, 1 cyc/row), and the gate-scaled
Wg matmuls run fp16 (4x cheaper than the fp32 path).
"""

import os
import sys

for _p in ("/opt/trn_rl_repo", "/root/.axon_site/_ro/trn_rl_repo"):
    if _p not in sys.path:
        sys.path.insert(0, _p)

import numpy as np

import concourse.bass as bass
import concourse.bacc as bacc
import concourse.mybir as mybir
import concourse.tile as tile
from concourse.bass_utils import run_bass_kernel_spmd
from concourse.masks import make_identity

f16 = np.float16
F32 = mybir.dt.float32
F16 = mybir.dt.float16
I16 = mybir.dt.int16
AF = mybir.ActivationFunctionType
ALU = mybir.AluOpType
AX = mybir.AxisListType

B, N, F, E, H = 8, 16384, 64, 262144, 128
SE_D = 32
FC1, FC2, OUT = 256, 128, 4
BN_EPS = 1e-3
NCORES = 8
NTILE = 128            # global 128-node dst tiles
TPC = NTILE // NCORES  # dst tiles per core (16)
BF = B * F             # 512, xt row width
MAX_GATHER = 512      # SWDGE ring limit: >1024 descriptors per gather crashes


def build_kernel(cpts, skip_collective: bool = False, phases: str = "GB"):
    """Build the SPMD program. cpts[i] = chunks (of 128 edges) for tile slot i
    (per-core tiles are sorted by descending edge count, so slot i's static
    size is the max of the i-th order statistic across cores)."""
    if isinstance(cpts, int):
        cpts = (cpts,) * TPC
    slots_i = [c * 128 for c in cpts]
    total_slots = sum(slots_i)
    offs_i = np.concatenate([[0], np.cumsum(slots_i)]).astype(int)
    nc = bacc.Bacc("TRN2", target_bir_lowering=False, debug=False,
                   num_devices=NCORES)

    # inputs (identical content on every core unless noted "per-core")
    xt = nc.dram_tensor("xt", [N, BF], F16, kind="ExternalInput")
    xs = nc.dram_tensor("xs", [N // NCORES, BF], F16, kind="ExternalInput")  # per-core x slice
    gidx = nc.dram_tensor("gidx", [128, total_slots // 16], I16, kind="ExternalInput")  # per-core
    # per-chunk (dloc, w) column pairs; S one-hot blocks are built on-chip
    dlw = nc.dram_tensor("dlw", [128, 2 * (total_slots // 128)], F32,
                         kind="ExternalInput")  # per-core
    iota16 = nc.dram_tensor("iota16", [128, 128], F16, kind="ExternalInput")
    bn1p = nc.dram_tensor("bn1p", [TPC, 128, 2], F32, kind="ExternalInput")             # per-core
    wpack = nc.dram_tensor("wpack", [128, 849], F32, kind="ExternalInput")
    bg4 = nc.dram_tensor("bg4", [1, BF], F16, kind="ExternalInput")  # bg tiled 4x

    out_t = nc.dram_tensor("out", [B, OUT], F32, kind="ExternalOutput")

    with tile.TileContext(nc) as tc:
        with (
            tc.tile_pool(name="const", bufs=1) as cpool,
            tc.tile_pool(name="sbuf", bufs=2) as pool,
            tc.tile_pool(name="psum", bufs=2, space="PSUM") as psum,
            tc.tile_pool(name="dram", bufs=1, space="DRAM") as dpool,
        ):
            # ---- constants / weights ----
            ident32 = cpool.tile([128, 128], F32)
            make_identity(nc, ident32[:])
            ident16 = cpool.tile([128, 128], F16)
            make_identity(nc, ident16[:])
            ones16 = cpool.tile([1, 128], F16)
            nc.vector.memset(ones16[:], 1.0)

            wpack_sb = cpool.tile([128, 849], F32)
            nc.sync.dma_start(out=wpack_sb[:], in_=wpack[:])
            bg4_sb = cpool.tile([1, BF], F16)
            nc.sync.dma_start(out=bg4_sb[:], in_=bg4[:])
            dlw_sb = cpool.tile([128, 2 * (total_slots // 128)], F32)
            nc.sync.dma_start(out=dlw_sb[:], in_=dlw[:])
            gidx_all = cpool.tile([128, total_slots // 16], I16)
            nc.sync.dma_start(out=gidx_all[:], in_=gidx[:])
            bn1all = cpool.tile([128, TPC, 2], F32)
            nc.sync.dma_start(out=bn1all[:],
                              in_=bn1p[:].rearrange("t p c -> p t c"))
            iota_sb = cpool.tile([128, 128], F16)
            nc.sync.dma_start(out=iota_sb[:], in_=iota16[:])
            wf1_sb = wpack_sb[:, 0:256]
            wf2a_sb = wpack_sb[:, 256:384]
            wf2b_sb = wpack_sb[:, 384:512]
            wg2_sb = wpack_sb[:, 512:640]
            wop2_sb = wpack_sb[0:SE_D, 640:768]
            w1_sb = wpack_sb[0:F, 768:800]
            w2_sb = wpack_sb[0:SE_D, 800:832]
            wo_sb = wpack_sb[:, 832:836]
            b1_sb = wpack_sb[0:SE_D, 836:837]
            b2_sb = wpack_sb[0:SE_D, 837:838]
            bop2_sb = wpack_sb[:, 838:839]
            bf1c_sb = wpack_sb[:, 839:841]
            bf2c_sb = wpack_sb[:, 841:842]
            g2_sb = wpack_sb[:, 842:844]
            be2_sb = wpack_sb[:, 844:846]
            g3_sb = wpack_sb[:, 846:847]
            be3_sb = wpack_sb[:, 847:848]
            boc_sb = wpack_sb[0:OUT, 848:849]

            # ---- phase G: SE gate (max-pool over nodes + tiny MLP) ----
            gates = None
            if "G" in phases:
                # sharded x-scan: each core reduces its N/8 slice, then
                # AllGather + local max of the [F, B] partials
                rows = N // NCORES
                half = rows // 2
                xs_sb = [cpool.tile([128, half // 128, BF], F16,
                                    tag=f"xs_sb{i}", name=f"xs_sb{i}")
                         for i in range(2)]
                reds = [cpool.tile([128, BF], F32, tag=f"reds{i}",
                                   name=f"reds{i}") for i in range(2)]
                for i in range(2):
                    nc.sync.dma_start(
                        out=xs_sb[i][:],
                        in_=xs[i * half:(i + 1) * half].rearrange(
                            "(p c) w -> p c w", p=128))
                    nc.vector.tensor_reduce(
                        out=reds[i][:],
                        in_=xs_sb[i][:].rearrange("p c w -> p w c"),
                        axis=AX.X, op=ALU.max,
                    )
                redpart = cpool.tile([128, BF], F32)
                nc.vector.tensor_tensor(out=redpart[:], in0=reds[0][:],
                                        in1=reds[1][:], op=ALU.max)
                pp = cpool.tile([F, B], F32)
                for b in range(B):
                    red_ps = psum.tile([F, 128], F32, space="PSUM",
                                       tag="ps_tr")
                    nc.tensor.transpose(
                        out=red_ps[:], in_=redpart[:, b * F:(b + 1) * F],
                        identity=ident32[:])
                    nc.vector.tensor_reduce(out=pp[:, b:b + 1], in_=red_ps[:],
                                            axis=AX.X, op=ALU.max)
                if skip_collective:
                    ppf = pp
                else:
                    r_in = dpool.tile([F, B], F32)
                    r_out = dpool.tile([NCORES, F, B], F32)
                    nc.gpsimd.dma_start(out=r_in[:], in_=pp[:])
                    nc.gpsimd.collective_compute(
                        "AllGather", ALU.bypass,
                        replica_groups=[list(range(NCORES))],
                        ins=[r_in.opt()], outs=[r_out.opt()],
                    )
                    ppg = cpool.tile([F, NCORES, B], F32)
                    nc.sync.dma_start(out=ppg[:],
                                      in_=r_out[:].rearrange("c p b -> p c b"))
                    ppf = cpool.tile([F, B], F32)
                    nc.vector.tensor_reduce(
                        out=ppf[:], in_=ppg[:].rearrange("p c b -> p b c"),
                        axis=AX.X, op=ALU.max)
                # gate MLP, all batches at once
                a1_ps = psum.tile([SE_D, B], F32, space="PSUM", tag="ps_b", bufs=1)
                nc.tensor.matmul(out=a1_ps[:], lhsT=w1_sb[:], rhs=ppf[:],
                                 start=True, stop=True)
                a1 = pool.tile([SE_D, B], F32, tag="a1")
                nc.scalar.activation(out=a1[:], in_=a1_ps[:], func=AF.Relu,
                                     bias=b1_sb[:])
                a2_ps = psum.tile([SE_D, B], F32, space="PSUM", tag="ps_b", bufs=1)
                nc.tensor.matmul(out=a2_ps[:], lhsT=w2_sb[:], rhs=a1[:],
                                 start=True, stop=True)
                a2 = pool.tile([SE_D, B], F32, tag="a2")
                nc.scalar.activation(out=a2[:], in_=a2_ps[:], func=AF.Relu,
                                     bias=b2_sb[:])
                g_ps = psum.tile([2 * F, B], F32, space="PSUM", tag="ps_b", bufs=1)
                nc.tensor.matmul(out=g_ps[:], lhsT=wop2_sb[:], rhs=a2[:],
                                 start=True, stop=True)
                gates = cpool.tile([2 * F, B], F32)
                nc.scalar.activation(out=gates[:], in_=g_ps[:],
                                     func=AF.Sigmoid, bias=bop2_sb[:])
                nc.vector.tensor_scalar_add(gates[:], gates[:], 1.0)

            # block-diagonal gate-scaled Wg pairs (fp16): for batch pair
            # (2j, 2j+1), rows 0:64 x cols 0:128 hold Wg*gate[2j] and rows
            # 64:128 x cols 128:256 hold Wg*gate[2j+1]. One h2 matmul then
            # computes both batches (contraction over the stacked f axis).
            wgebz = []
            for j in range(4):
                wz = cpool.tile([2 * F, 2 * H], F16, tag=f"wgebz_{j}",
                                name=f"wgebz_{j}")
                nc.vector.memset(wz[:], 0.0)
                if gates is not None:
                    nc.vector.tensor_scalar(
                        out=wz[0:F, 0:H], in0=wg2_sb[0:F, :],
                        scalar1=gates[0:F, 2 * j:2 * j + 1],
                        scalar2=None, op0=ALU.mult)
                    nc.vector.tensor_scalar(
                        out=wz[F:2 * F, H:2 * H], in0=wg2_sb[F:2 * F, :],
                        scalar1=gates[F:2 * F, 2 * j + 1:2 * j + 2],
                        scalar2=None, op0=ALU.mult)
                else:
                    nc.vector.tensor_copy(out=wz[0:F, 0:H],
                                          in_=wg2_sb[0:F, :])
                    nc.vector.tensor_copy(out=wz[F:2 * F, H:2 * H],
                                          in_=wg2_sb[F:2 * F, :])
                wgebz.append(wz)

            # ---- phase B: SpMM + Wg + BN1 + pool partial ----
            pooled = cpool.tile([H, B], F32)
            poolacc = [cpool.tile([128, BF], F16, tag=f"poolacc_{g}",
                                  name=f"poolacc_{g}")
                       for g in range(2)]

            for t in range(TPC) if "B" in phases else []:
                cpt_t = cpts[t]
                slots_t = slots_i[t]
                off_t = int(offs_i[t])

                # build the one-hot S blocks on-chip: row e of chunk k gets
                # w at column dloc ((iota == dloc) * w), zeros elsewhere
                s_sb = pool.tile([128, slots_t], F16, tag="s_sb", bufs=4,
                                 name="s_sb")
                cbase = off_t // 128
                for k in range(cpt_t):
                    c = cbase + k
                    nc.vector.tensor_scalar(
                        out=s_sb[:, k * 128:(k + 1) * 128], in0=iota_sb[:],
                        scalar1=dlw_sb[:, 2 * c:2 * c + 1],
                        scalar2=dlw_sb[:, 2 * c + 1:2 * c + 2],
                        op0=ALU.is_equal, op1=ALU.mult)

                msg = pool.tile([128, cpt_t, BF], F16, tag="msg", bufs=5,
                                name="msg")
                for s0 in range(0, slots_t, MAX_GATHER):
                    n_i = min(MAX_GATHER, slots_t - s0)
                    nc.gpsimd.dma_gather(
                        out_ap=msg[:, s0 // 128:(s0 + n_i) // 128, :],
                        in_ap=xt[:],
                        idxs_ap=gidx_all[:, (off_t + s0) // 16:(off_t + s0 + n_i) // 16],
                        num_idxs=n_i, num_idxs_reg=n_i, elem_size=BF,
                    )
                agg_ps = psum.tile([128, BF], F32, space="PSUM", tag="ps_agg",
                                   bufs=2)
                for k in range(cpt_t):
                    nc.tensor.matmul(
                        out=agg_ps[:],
                        lhsT=s_sb[:, k * 128:(k + 1) * 128],
                        rhs=msg[:, k, :],
                        start=(k == 0), stop=(k == cpt_t - 1),
                    )
                agg32 = pool.tile([128, BF], F32, tag="agg32", bufs=3,
                                  name="agg32")
                nc.scalar.activation(out=agg32[:], in_=agg_ps[:], func=AF.Copy)
                # batch-pair transposes; aggT[:, j, :] holds batches 2j
                # (partitions 0:64) and 2j+1 (64:128), fp16-cast on evacuation
                aggT = pool.tile([128, 4, 128], F16, tag="aggT", bufs=6,
                                 name="aggT")
                for j in range(4):
                    tr_ps = psum.tile([128, 128], F32, space="PSUM",
                                      tag="ps_tr")
                    nc.tensor.transpose(
                        out=tr_ps[:], in_=agg32[:, j * 128:(j + 1) * 128],
                        identity=ident32[:])
                    eng = nc.vector if j % 2 == 0 else nc.scalar
                    if eng is nc.vector:
                        eng.tensor_copy(out=aggT[:, j, :], in_=tr_ps[:])
                    else:
                        eng.activation(out=aggT[:, j, :], in_=tr_ps[:],
                                       func=AF.Copy)
                # h2 = relu(agg_gated @ Wg + bg), 4 batches per PSUM bank via
                # block-diagonal pair weights
                sums = pool.tile([128, 2], F32, tag="sums", bufs=4, name="sums")
                sqs = pool.tile([128, 2], F32, tag="sqs", bufs=4, name="sqs")
                h2g = []
                for g in range(2):
                    h2_ps = psum.tile([128, BF], F32, space="PSUM",
                                      tag="ps_h2", bufs=3)
                    nc.tensor.matmul(out=h2_ps[:], lhsT=ones16[:],
                                     rhs=bg4_sb[:], start=True, stop=False)
                    for jj in range(2):
                        j = g * 2 + jj
                        nc.tensor.matmul(
                            out=h2_ps[:, jj * 2 * H:(jj + 1) * 2 * H],
                            lhsT=aggT[:, j, :],
                            rhs=wgebz[j][:],
                            start=False, stop=(jj == 1))
                    h2 = pool.tile([128, BF], F16, tag=f"h2_{g}", bufs=3,
                                   name=f"h2_{g}")
                    nc.scalar.activation(out=h2[:], in_=h2_ps[:], func=AF.Relu,
                                         accum_out=sums[:, g:g + 1])
                    sqscr = pool.tile([128, BF], F16, tag="sqscr", bufs=3,
                                      name="sqscr")
                    nc.vector.tensor_tensor(out=sqscr[:], in0=h2[:],
                                            in1=h2[:], op=ALU.mult)
                    nc.vector.tensor_reduce(out=sqs[:, g:g + 1],
                                            in_=sqscr[:], axis=AX.X,
                                            op=ALU.add)
                    h2g.append(h2)
                # BN1 per-node affine coefficients (DVE column math)
                rsumt = pool.tile([128, 1], F32, tag="rsumt", bufs=4, name="rsumt")
                nc.vector.tensor_reduce(out=rsumt[:], in_=sums[:], axis=AX.X,
                                        op=ALU.add)
                sqsumt = pool.tile([128, 1], F32, tag="sqsumt", bufs=4, name="sqsumt")
                nc.vector.tensor_reduce(out=sqsumt[:], in_=sqs[:], axis=AX.X,
                                        op=ALU.add)
                mean = pool.tile([128, 1], F32, tag="mean", bufs=4, name="mean")
                nc.vector.tensor_scalar_mul(mean[:], rsumt[:], 1.0 / (B * H))
                msqe = pool.tile([128, 1], F32, tag="msqe", bufs=4, name="msqe")
                nc.vector.tensor_scalar_mul(msqe[:], sqsumt[:], 1.0 / (B * H))
                nc.vector.tensor_scalar_add(msqe[:], msqe[:], BN_EPS)
                var = pool.tile([128, 1], F32, tag="var", bufs=4, name="var")
                nc.vector.tensor_tensor(out=var[:], in0=mean[:], in1=mean[:],
                                        op=ALU.mult)
                nc.vector.tensor_tensor(out=var[:], in0=msqe[:], in1=var[:],
                                        op=ALU.subtract)
                inv = pool.tile([128, 1], F32, tag="inv", bufs=4, name="inv")
                nc.vector.reciprocal(out=inv[:], in_=var[:])
                rstd = pool.tile([128, 1], F32, tag="rstd", bufs=4, name="rstd")
                nc.scalar.sqrt(out=rstd[:], in_=inv[:])
                aco = pool.tile([128, 1], F32, tag="aco", bufs=4, name="aco")
                nc.vector.tensor_tensor(out=aco[:], in0=rstd[:],
                                        in1=bn1all[:, t, 0:1], op=ALU.mult)
                bco = pool.tile([128, 1], F32, tag="bco", bufs=4, name="bco")
                nc.vector.tensor_tensor(out=bco[:], in0=mean[:], in1=aco[:],
                                        op=ALU.mult)
                nc.vector.tensor_tensor(out=bco[:], in0=bn1all[:, t, 1:2],
                                        in1=bco[:], op=ALU.subtract)
                # apply BN1 (ACT: out = aco*h2 + bco) and fold into pool max
                for g in range(2):
                    if t == 0:
                        nc.vector.tensor_scalar(
                            out=poolacc[g][:], in0=h2g[g][:],
                            scalar1=aco[:, 0:1], scalar2=bco[:, 0:1],
                            op0=ALU.mult, op1=ALU.add)
                    else:
                        h2n = pool.tile([128, BF], F16, tag="h2n", bufs=3,
                                        name="h2n")
                        nc.vector.tensor_scalar(
                            out=h2n[:], in0=h2g[g][:],
                            scalar1=aco[:, 0:1], scalar2=bco[:, 0:1],
                            op0=ALU.mult, op1=ALU.add)
                        nc.vector.tensor_tensor(out=poolacc[g][:],
                                                in0=poolacc[g][:], in1=h2n[:],
                                                op=ALU.max)

            # fold pooled partials: per batch, transpose + reduce over nodes
            pacc32 = [cpool.tile([128, BF], F32, tag=f"pacc32_{g}",
                                 name=f"pacc32_{g}") for g in range(2)]
            for g in range(2):
                nc.scalar.activation(out=pacc32[g][:], in_=poolacc[g][:],
                                     func=AF.Copy)
            for b in range(B):
                g, j = b // 4, b % 4
                hT_ps = psum.tile([128, 128], F32, space="PSUM", tag="ps_tr")
                nc.tensor.transpose(
                    out=hT_ps[:], in_=pacc32[g][:, j * H:(j + 1) * H],
                    identity=ident32[:])
                nc.vector.tensor_reduce(out=pooled[:, b:b + 1], in_=hT_ps[:],
                                        axis=AX.X, op=ALU.max)

            # ---- phase C: AllGather + local max + replicated head ----
            if skip_collective:
                pooledf = pooled
            else:
                p_in = dpool.tile([H, B], F32)
                p_out = dpool.tile([NCORES, H, B], F32)
                nc.gpsimd.dma_start(out=p_in[:], in_=pooled[:])
                nc.gpsimd.collective_compute(
                    "AllGather", ALU.bypass,
                    replica_groups=[list(range(NCORES))],
                    ins=[p_in.opt()], outs=[p_out.opt()],
                )
                pg = cpool.tile([H, NCORES, B], F32)
                nc.sync.dma_start(out=pg[:],
                                  in_=p_out[:].rearrange("c p b -> p c b"))
                pooledf = cpool.tile([H, B], F32)
                nc.vector.tensor_reduce(
                    out=pooledf[:], in_=pg[:].rearrange("p c b -> p b c"),
                    axis=AX.X, op=ALU.max)

            def bn_free8(z, nrows, gamma_col, beta_col, tag):
                """BN over the 8 free-dim entries of z [nrows, 8] -> new tile."""
                rs = pool.tile([nrows, 1], F32, tag=f"{tag}_rs")
                nc.vector.tensor_reduce(out=rs[:], in_=z[:], axis=AX.X,
                                        op=ALU.add)
                nc.vector.tensor_scalar_mul(rs[:], rs[:], 1.0 / B)
                sqt = pool.tile([nrows, B], F32, tag=f"{tag}_sqt")
                sq = pool.tile([nrows, 1], F32, tag=f"{tag}_sq")
                nc.scalar.activation(out=sqt[:], in_=z[:], func=AF.Square,
                                     accum_out=sq[:])
                nc.vector.tensor_scalar_mul(sq[:], sq[:], 1.0 / B)
                v = pool.tile([nrows, 1], F32, tag=f"{tag}_v")
                nc.vector.tensor_tensor(out=v[:], in0=rs[:], in1=rs[:],
                                        op=ALU.mult)
                nc.vector.tensor_tensor(out=v[:], in0=sq[:], in1=v[:],
                                        op=ALU.subtract)
                nc.vector.tensor_scalar_add(v[:], v[:], BN_EPS)
                iv = pool.tile([nrows, 1], F32, tag=f"{tag}_iv")
                nc.vector.reciprocal(out=iv[:], in_=v[:])
                rst = pool.tile([nrows, 1], F32, tag=f"{tag}_rst")
                nc.scalar.sqrt(out=rst[:], in_=iv[:])
                ac = pool.tile([nrows, 1], F32, tag=f"{tag}_ac")
                nc.vector.tensor_tensor(out=ac[:], in0=rst[:], in1=gamma_col,
                                        op=ALU.mult)
                bc = pool.tile([nrows, 1], F32, tag=f"{tag}_bc")
                nc.vector.tensor_tensor(out=bc[:], in0=rs[:], in1=ac[:],
                                        op=ALU.mult)
                nc.vector.tensor_tensor(out=bc[:], in0=beta_col, in1=bc[:],
                                        op=ALU.subtract)
                zn = pool.tile([nrows, B], F32, tag=f"{tag}_zn")
                nc.vector.tensor_scalar(out=zn[:], in0=z[:],
                                        scalar1=ac[:, 0:1], scalar2=bc[:, 0:1],
                                        op0=ALU.mult, op1=ALU.add)
                return zn

            # head in feature-major layout: features on partitions, batch on
            # free. Biases ride ACT's per-partition bias port; no transposes.
            z1n = []
            for j in range(2):
                z1_ps = psum.tile([H, B], F32, space="PSUM", tag="ps_b", bufs=1)
                nc.tensor.matmul(out=z1_ps[:],
                                 lhsT=wf1_sb[:, j * 128:(j + 1) * 128],
                                 rhs=pooledf[:], start=True, stop=True)
                z1T = pool.tile([H, B], F32, tag=f"z1T_{j}")
                nc.scalar.activation(out=z1T[:], in_=z1_ps[:], func=AF.Relu,
                                     bias=bf1c_sb[:, j:j + 1])
                z1n.append(bn_free8(z1T, 128, g2_sb[:, j:j + 1],
                                    be2_sb[:, j:j + 1], f"bn2_{j}"))
            z2_ps = psum.tile([FC2, B], F32, space="PSUM", tag="ps_b", bufs=1)
            nc.tensor.matmul(out=z2_ps[:], lhsT=wf2a_sb[:], rhs=z1n[0][:],
                             start=True, stop=False)
            nc.tensor.matmul(out=z2_ps[:], lhsT=wf2b_sb[:], rhs=z1n[1][:],
                             start=False, stop=True)
            z2T = pool.tile([FC2, B], F32, tag="z2T")
            nc.scalar.activation(out=z2T[:], in_=z2_ps[:], func=AF.Relu,
                                 bias=bf2c_sb[:])
            z2n = bn_free8(z2T, FC2, g3_sb[:, 0:1], be3_sb[:, 0:1], "bn3")
            # logits^T [OUT, B], add bias, transpose to [B, OUT], softmax
            lg_ps = psum.tile([OUT, B], F32, space="PSUM", tag="ps_b", bufs=1)
            nc.tensor.matmul(out=lg_ps[:], lhsT=wo_sb[:], rhs=z2n[:],
                             start=True, stop=True)
            lgT = pool.tile([OUT, B], F32, tag="lgT")
            nc.vector.tensor_scalar(out=lgT[:], in0=lg_ps[:],
                                    scalar1=boc_sb[:, 0:1], scalar2=None,
                                    op0=ALU.add)
            lgt_ps = psum.tile([B, OUT], F32, space="PSUM", tag="ps_b", bufs=1)
            nc.tensor.transpose(out=lgt_ps[:], in_=lgT[:],
                                identity=ident32[0:OUT, 0:OUT])
            lg = pool.tile([B, OUT], F32, tag="lg")
            nc.vector.tensor_copy(out=lg[:], in_=lgt_ps[:])
            mx = pool.tile([B, 1], F32, tag="mx")
            nc.vector.tensor_reduce(out=mx[:], in_=lg[:], axis=AX.X, op=ALU.max)
            ex = pool.tile([B, OUT], F32, tag="ex")
            nc.vector.tensor_scalar(out=ex[:], in0=lg[:], scalar1=mx[:, 0:1],
                                    scalar2=None, op0=ALU.subtract)
            nc.scalar.activation(out=ex[:], in_=ex[:], func=AF.Exp)
            ssum = pool.tile([B, 1], F32, tag="ssum")
            nc.vector.tensor_reduce(out=ssum[:], in_=ex[:], axis=AX.X,
                                    op=ALU.add)
            sinv = pool.tile([B, 1], F32, tag="sinv")
            nc.vector.reciprocal(out=sinv[:], in_=ssum[:])
            sm = pool.tile([B, OUT], F32, tag="sm")
            nc.vector.tensor_scalar(out=sm[:], in0=ex[:], scalar1=sinv[:, 0:1],
                                    scalar2=None, op0=ALU.mult)
            nc.sync.dma_start(out=out_t[:], in_=sm[:])
    nc.compile()
    return nc


def preprocess(x, src, dst, edge_w):
    """Host marshalling: node-major x table + sorted/padded edge tiles."""
    order = np.argsort(dst, kind="stable")
    ss = src[order].astype(np.int64)
    ds = dst[order].astype(np.int64)
    ws = edge_w[order].astype(np.float32)
    tile_id = ds // 128
    dloc = ds % 128
    counts = np.bincount(tile_id, minlength=NTILE)
    cpt = int(np.ceil(counts.max() / 128))
    slots = cpt * 128

    gidx_all = np.zeros((NTILE, slots), np.int16)
    sval = np.zeros((NTILE, slots), np.float32)
    sloc = np.zeros((NTILE, slots), np.int64)
    offs = np.concatenate([[0], np.cumsum(counts)])
    for t in range(NTILE):
        cnt = counts[t]
        seg = slice(offs[t], offs[t + 1])
        # order each tile's edges by ascending src so gather descriptors
        # read ascending HBM addresses (DRAM row-buffer locality); the
        # segment-sum is order-invariant since S follows the slot order
        o = np.argsort(ss[seg], kind="stable")
        gidx_all[t, :cnt] = ss[seg][o]
        sval[t, :cnt] = ws[seg][o]
        sloc[t, :cnt] = dloc[seg][o]

    # per-chunk (dloc, w) columns; padded slots get dloc=200 (unmatchable)
    dlocw = np.zeros((NTILE, 128, 2 * cpt), np.float32)
    for t in range(NTILE):
        dl = sloc[t].reshape(cpt, 128).T.astype(np.float32)
        wv = sval[t].reshape(cpt, 128).T
        dl[wv == 0.0] = 200.0
        dlocw[t, :, 0::2] = dl
        dlocw[t, :, 1::2] = wv
    dlocw = dlocw.astype(np.float32)

    # wrapped int16 index tables: [16, slots//16] replicated to 128 partitions
    gidx_w = np.zeros((NTILE, 128, slots // 16), np.int16)
    for t in range(NTILE):
        base = gidx_all[t].reshape(slots // 16, 16).T
        gidx_w[t] = np.tile(base, (8, 1))

    # per-core tile order (descending edge count) and per-slot chunk counts
    order_pc = np.zeros((NCORES, TPC), np.int64)
    for c in range(NCORES):
        tl = np.arange(c * TPC, (c + 1) * TPC)
        order_pc[c] = tl[np.argsort(-counts[tl], kind="stable")]
    cpts = tuple(
        int(np.ceil(max(counts[order_pc[c][i]] for c in range(NCORES)) / 128))
        for i in range(TPC)
    )
    xt = np.ascontiguousarray(
        np.asarray(x, np.float32).transpose(1, 0, 2).reshape(N, BF)
    ).astype(f16)
    return xt, gidx_w, dlocw, cpts, order_pc


def make_in_maps(inputs, xt, gidx_w, dlocw, cpts, order_pc):
    g1 = np.asarray(inputs["g1"], np.float32).reshape(NTILE, 128)
    beta1 = np.asarray(inputs["beta1"], np.float32).reshape(NTILE, 128)
    bn1 = np.stack([g1, beta1], axis=-1)  # [NTILE, 128, 2]

    f32 = lambda a: np.ascontiguousarray(np.asarray(a, np.float32))
    wp = np.zeros((128, 849), np.float32)
    wp[:, 0:256] = f32(inputs["Wf1"])
    wf2 = f32(inputs["Wf2"]).reshape(2, H, FC2)
    wp[:, 256:384] = wf2[0]
    wp[:, 384:512] = wf2[1]
    wp[:, 512:640] = np.tile(f32(inputs["Wg"]), (2, 1))
    wp[0:SE_D, 640:768] = np.concatenate([f32(inputs["Wop"])] * 2, axis=1)
    wp[0:F, 768:800] = f32(inputs["W1"])
    wp[0:SE_D, 800:832] = f32(inputs["W2"])
    wp[:, 832:836] = f32(inputs["Wo"])
    wp[0:SE_D, 836] = f32(inputs["b1"])
    wp[0:SE_D, 837] = f32(inputs["b2"])
    wp[:, 838] = np.tile(f32(inputs["bop"]), 2)
    wp[:, 839:841] = f32(inputs["bf1"]).reshape(2, H).T
    wp[:, 841] = f32(inputs["bf2"])
    wp[:, 842:844] = f32(inputs["g2"]).reshape(2, H).T
    wp[:, 844:846] = f32(inputs["beta2"]).reshape(2, H).T
    wp[:, 846] = f32(inputs["g3"])
    wp[:, 847] = f32(inputs["beta3"])
    wp[0:OUT, 848] = f32(inputs["bo"])
    shared = {
        "xt": xt,
        "iota16": np.tile(np.arange(128, dtype=np.float32), (128, 1)).astype(f16),
        "wpack": wp,
        "bg4": np.tile(f32(inputs["bg"]).reshape(1, H), (1, 4)).astype(f16),
    }
    in_maps = []
    for c in range(NCORES):
        order = order_pc[c]
        m = dict(shared)
        m["xs"] = np.ascontiguousarray(xt[c * (N // NCORES):(c + 1) * (N // NCORES)])
        m["gidx"] = np.ascontiguousarray(np.concatenate(
            [gidx_w[gt][:, :cpts[i] * 8] for i, gt in enumerate(order)], axis=1))
        m["dlw"] = np.ascontiguousarray(np.concatenate(
            [dlocw[gt][:, :2 * cpts[i]] for i, gt in enumerate(order)], axis=1))
        m["bn1p"] = np.ascontiguousarray(bn1[order])
        in_maps.append(m)
    return in_maps


_CACHE = {}
LAST_RESULT = None  # BassKernelResults of the most recent kernel() call


def kernel(**inputs):
    global LAST_RESULT
    xt, gidx_w, dlocw, cpts, order_pc = preprocess(
        np.asarray(inputs["x"]), np.asarray(inputs["src"]),
        np.asarray(inputs["dst"]), np.asarray(inputs["edge_w"]))
    if cpts not in _CACHE:
        _CACHE[cpts] = build_kernel(cpts)
    nc = _CACHE[cpts]
    in_maps = make_in_maps(inputs, xt, gidx_w, dlocw, cpts, order_pc)
    trace = os.environ.get("BASS_KERNEL_TRACE", "0") == "1"
    res = run_bass_kernel_spmd(nc, in_maps, list(range(NCORES)), trace=trace)
    LAST_RESULT = res
    return np.asarray(res.results[0]["out"], np.float32)


# revision 45
# speedup vs baseline: 1.0138x; 1.0019x over previous
"""Trainium2 Bass kernel for nn_BaseGCNModel_addSE (gnn_message_passing).

SPMD over 8 NeuronCores, data laid out so the SE gate commutes with the
sparse aggregation:

    agg = A @ (x * (1+gate)) = (A @ x) * (1+gate)

since the gate is constant along the contracted node axis. The kernel
gathers messages directly from the host-marshalled node-major table
xt [N, B*F] (fp16, 1KB rows), segment-sums them on the PE via streamed
fp16 one-hot blocks (edge weights folded in), and applies the gate by
scaling Wg per batch. Every core owns 16 of the 128 dst-node tiles; BN1
is node-local; pooled partials are combined with AllGather + local max
(cheaper than AllReduce in the fabric); the FC head runs replicated in
feature-major layout (no transposes, per-partition bias/scale on ACT).

Mid-pipeline runs in fp16: agg is evacuated to fp16, transposed in
128x128 pairs on the PE (fp16 identity, 1 cyc/row), and the gate-scaled
Wg matmuls run fp16 (4x cheaper than the fp32 path).
"""

import os
import sys

for _p in ("/opt/trn_rl_repo", "/root/.axon_site/_ro/trn_rl_repo"):
    if _p not in sys.path:
        sys.path.insert(0, _p)

import numpy as np

import concourse.bass as bass
import concourse.bacc as bacc
import concourse.mybir as mybir
import concourse.tile as tile
from concourse.bass_utils import run_bass_kernel_spmd
from concourse.masks import make_identity

f16 = np.float16
F32 = mybir.dt.float32
F16 = mybir.dt.float16
I16 = mybir.dt.int16
AF = mybir.ActivationFunctionType
ALU = mybir.AluOpType
AX = mybir.AxisListType

B, N, F, E, H = 8, 16384, 64, 262144, 128
SE_D = 32
FC1, FC2, OUT = 256, 128, 4
BN_EPS = 1e-3
NCORES = 8
NTILE = 128            # global 128-node dst tiles
TPC = NTILE // NCORES  # dst tiles per core (16)
BF = B * F             # 512, xt row width
MAX_GATHER = 512      # SWDGE ring limit: >1024 descriptors per gather crashes


def build_kernel(cpts, skip_collective: bool = False, phases: str = "GB"):
    """Build the SPMD program. cpts[i] = chunks (of 128 edges) for tile slot i
    (per-core tiles are sorted by descending edge count, so slot i's static
    size is the max of the i-th order statistic across cores)."""
    if isinstance(cpts, int):
        cpts = (cpts,) * TPC
    slots_i = [c * 128 for c in cpts]
    total_slots = sum(slots_i)
    offs_i = np.concatenate([[0], np.cumsum(slots_i)]).astype(int)
    nc = bacc.Bacc("TRN2", target_bir_lowering=False, debug=False,
                   num_devices=NCORES)

    # inputs (identical content on every core unless noted "per-core")
    xt = nc.dram_tensor("xt", [N, BF], F16, kind="ExternalInput")
    xs = nc.dram_tensor("xs", [N // NCORES, BF], F16, kind="ExternalInput")  # per-core x slice
    gidx = nc.dram_tensor("gidx", [128, total_slots // 16], I16, kind="ExternalInput")  # per-core
    # per-chunk (dloc, w) column pairs; S one-hot blocks are built on-chip
    dlw = nc.dram_tensor("dlw", [128, 2 * (total_slots // 128)], F32,
                         kind="ExternalInput")  # per-core
    iota16 = nc.dram_tensor("iota16", [128, 128], F16, kind="ExternalInput")
    bn1p = nc.dram_tensor("bn1p", [TPC, 128, 2], F32, kind="ExternalInput")             # per-core
    wpack = nc.dram_tensor("wpack", [128, 849], F32, kind="ExternalInput")
    bg4 = nc.dram_tensor("bg4", [1, BF], F16, kind="ExternalInput")  # bg tiled 4x

    out_t = nc.dram_tensor("out", [B, OUT], F32, kind="ExternalOutput")

    with tile.TileContext(nc) as tc:
        with (
            tc.tile_pool(name="const", bufs=1) as cpool,
            tc.tile_pool(name="sbuf", bufs=2) as pool,
            tc.tile_pool(name="psum", bufs=2, space="PSUM") as psum,
            tc.tile_pool(name="dram", bufs=1, space="DRAM") as dpool,
        ):
            # ---- constants / weights ----
            ident32 = cpool.tile([128, 128], F32)
            make_identity(nc, ident32[:])
            ident16 = cpool.tile([128, 128], F16)
            make_identity(nc, ident16[:])
            ones16 = cpool.tile([1, 128], F16)
            nc.vector.memset(ones16[:], 1.0)

            wpack_sb = cpool.tile([128, 849], F32)
            nc.sync.dma_start(out=wpack_sb[:], in_=wpack[:])
            bg4_sb = cpool.tile([1, BF], F16)
            nc.sync.dma_start(out=bg4_sb[:], in_=bg4[:])
            dlw_sb = cpool.tile([128, 2 * (total_slots // 128)], F32)
            nc.sync.dma_start(out=dlw_sb[:], in_=dlw[:])
            gidx_all = cpool.tile([128, total_slots // 16], I16)
            nc.sync.dma_start(out=gidx_all[:], in_=gidx[:])
            bn1all = cpool.tile([128, TPC, 2], F32)
            nc.sync.dma_start(out=bn1all[:],
                              in_=bn1p[:].rearrange("t p c -> p t c"))
            iota_sb = cpool.tile([128, 128], F16)
            nc.sync.dma_start(out=iota_sb[:], in_=iota16[:])
            wf1_sb = wpack_sb[:, 0:256]
            wf2a_sb = wpack_sb[:, 256:384]
            wf2b_sb = wpack_sb[:, 384:512]
            wg2_sb = wpack_sb[:, 512:640]
            wop2_sb = wpack_sb[0:SE_D, 640:768]
            w1_sb = wpack_sb[0:F, 768:800]
            w2_sb = wpack_sb[0:SE_D, 800:832]
            wo_sb = wpack_sb[:, 832:836]
            b1_sb = wpack_sb[0:SE_D, 836:837]
            b2_sb = wpack_sb[0:SE_D, 837:838]
            bop2_sb = wpack_sb[:, 838:839]
            bf1c_sb = wpack_sb[:, 839:841]
            bf2c_sb = wpack_sb[:, 841:842]
            g2_sb = wpack_sb[:, 842:844]
            be2_sb = wpack_sb[:, 844:846]
            g3_sb = wpack_sb[:, 846:847]
            be3_sb = wpack_sb[:, 847:848]
            boc_sb = wpack_sb[0:OUT, 848:849]

            # ---- phase G: SE gate (max-pool over nodes + tiny MLP) ----
            gates = None
            if "G" in phases:
                # sharded x-scan: each core reduces its N/8 slice, then
                # AllGather + local max of the [F, B] partials
                rows = N // NCORES
                half = rows // 2
                xs_sb = [cpool.tile([128, half // 128, BF], F16,
                                    tag=f"xs_sb{i}", name=f"xs_sb{i}")
                         for i in range(2)]
                reds = [cpool.tile([128, BF], F32, tag=f"reds{i}",
                                   name=f"reds{i}") for i in range(2)]
                for i in range(2):
                    nc.sync.dma_start(
                        out=xs_sb[i][:],
                        in_=xs[i * half:(i + 1) * half].rearrange(
                            "(p c) w -> p c w", p=128))
                    nc.vector.tensor_reduce(
                        out=reds[i][:],
                        in_=xs_sb[i][:].rearrange("p c w -> p w c"),
                        axis=AX.X, op=ALU.max,
                    )
                redpart = cpool.tile([128, BF], F32)
                nc.vector.tensor_tensor(out=redpart[:], in0=reds[0][:],
                                        in1=reds[1][:], op=ALU.max)
                pp = cpool.tile([F, B], F32)
                for b in range(B):
                    red_ps = psum.tile([F, 128], F32, space="PSUM",
                                       tag="ps_tr")
                    nc.tensor.transpose(
                        out=red_ps[:], in_=redpart[:, b * F:(b + 1) * F],
                        identity=ident32[:])
                    nc.vector.tensor_reduce(out=pp[:, b:b + 1], in_=red_ps[:],
                                            axis=AX.X, op=ALU.max)
                if skip_collective:
                    ppf = pp
                else:
                    r_in = dpool.tile([F, B], F32)
                    r_out = dpool.tile([NCORES, F, B], F32)
                    nc.gpsimd.dma_start(out=r_in[:], in_=pp[:])
                    nc.gpsimd.collective_compute(
                        "AllGather", ALU.bypass,
                        replica_groups=[list(range(NCORES))],
                        ins=[r_in.opt()], outs=[r_out.opt()],
                    )
                    ppg = cpool.tile([F, NCORES, B], F32)
                    nc.sync.dma_start(out=ppg[:],
                                      in_=r_out[:].rearrange("c p b -> p c b"))
                    ppf = cpool.tile([F, B], F32)
                    nc.vector.tensor_reduce(
                        out=ppf[:], in_=ppg[:].rearrange("p c b -> p b c"),
                        axis=AX.X, op=ALU.max)
                # gate MLP, all batches at once
                a1_ps = psum.tile([SE_D, B], F32, space="PSUM", tag="ps_b", bufs=1)
                nc.tensor.matmul(out=a1_ps[:], lhsT=w1_sb[:], rhs=ppf[:],
                                 start=True, stop=True)
                a1 = pool.tile([SE_D, B], F32, tag="a1")
                nc.scalar.activation(out=a1[:], in_=a1_ps[:], func=AF.Relu,
                                     bias=b1_sb[:])
                a2_ps = psum.tile([SE_D, B], F32, space="PSUM", tag="ps_b", bufs=1)
                nc.tensor.matmul(out=a2_ps[:], lhsT=w2_sb[:], rhs=a1[:],
                                 start=True, stop=True)
                a2 = pool.tile([SE_D, B], F32, tag="a2")
                nc.scalar.activation(out=a2[:], in_=a2_ps[:], func=AF.Relu,
                                     bias=b2_sb[:])
                g_ps = psum.tile([2 * F, B], F32, space="PSUM", tag="ps_b", bufs=1)
                nc.tensor.matmul(out=g_ps[:], lhsT=wop2_sb[:], rhs=a2[:],
                                 start=True, stop=True)
                gates = cpool.tile([2 * F, B], F32)
                nc.scalar.activation(out=gates[:], in_=g_ps[:],
                                     func=AF.Sigmoid, bias=bop2_sb[:])
                nc.vector.tensor_scalar_add(gates[:], gates[:], 1.0)

            # block-diagonal gate-scaled Wg pairs (fp16): for batch pair
            # (2j, 2j+1), rows 0:64 x cols 0:128 hold Wg*gate[2j] and rows
            # 64:128 x cols 128:256 hold Wg*gate[2j+1]. One h2 matmul then
            # computes both batches (contraction over the stacked f axis).
            wgebz = []
            for j in range(4):
                wz = cpool.tile([2 * F, 2 * H], F16, tag=f"wgebz_{j}",
                                name=f"wgebz_{j}")
                nc.vector.memset(wz[:], 0.0)
                if gates is not None:
                    nc.vector.tensor_scalar(
                        out=wz[0:F, 0:H], in0=wg2_sb[0:F, :],
                        scalar1=gates[0:F, 2 * j:2 * j + 1],
                        scalar2=None, op0=ALU.mult)
                    nc.vector.tensor_scalar(
                        out=wz[F:2 * F, H:2 * H], in0=wg2_sb[F:2 * F, :],
                        scalar1=gates[F:2 * F, 2 * j + 1:2 * j + 2],
                        scalar2=None, op0=ALU.mult)
                else:
                    nc.vector.tensor_copy(out=wz[0:F, 0:H],
                                          in_=wg2_sb[0:F, :])
                    nc.vector.tensor_copy(out=wz[F:2 * F, H:2 * H],
                                          in_=wg2_sb[F:2 * F, :])
                wgebz.append(wz)

            # ---- phase B: SpMM + Wg + BN1 + pool partial ----
            pooled = cpool.tile([H, B], F32)
            poolacc = [cpool.tile([128, BF], F16, tag=f"poolacc_{g}",
                                  name=f"poolacc_{g}")
                       for g in range(2)]

            for t in range(TPC) if "B" in phases else []:
                cpt_t = cpts[t]
                slots_t = slots_i[t]
                off_t = int(offs_i[t])

                # build the one-hot S blocks on-chip: row e of chunk k gets
                # w at column dloc ((iota == dloc) * w), zeros elsewhere
                s_sb = pool.tile([128, slots_t], F16, tag="s_sb", bufs=4,
                                 name="s_sb")
                cbase = off_t // 128
                for k in range(cpt_t):
                    c = cbase + k
                    nc.vector.tensor_scalar(
                        out=s_sb[:, k * 128:(k + 1) * 128], in0=iota_sb[:],
                        scalar1=dlw_sb[:, 2 * c:2 * c + 1],
                        scalar2=dlw_sb[:, 2 * c + 1:2 * c + 2],
                        op0=ALU.is_equal, op1=ALU.mult)

                msg = pool.tile([128, cpt_t, BF], F16, tag="msg", bufs=5,
                                name="msg")
                for s0 in range(0, slots_t, MAX_GATHER):
                    n_i = min(MAX_GATHER, slots_t - s0)
                    nc.gpsimd.dma_gather(
                        out_ap=msg[:, s0 // 128:(s0 + n_i) // 128, :],
                        in_ap=xt[:],
                        idxs_ap=gidx_all[:, (off_t + s0) // 16:(off_t + s0 + n_i) // 16],
                        num_idxs=n_i, num_idxs_reg=n_i, elem_size=BF,
                    )
                agg_ps = psum.tile([128, BF], F32, space="PSUM", tag="ps_agg",
                                   bufs=2)
                for k in range(cpt_t):
                    nc.tensor.matmul(
                        out=agg_ps[:],
                        lhsT=s_sb[:, k * 128:(k + 1) * 128],
                        rhs=msg[:, k, :],
                        start=(k == 0), stop=(k == cpt_t - 1),
                    )
                agg32 = pool.tile([128, BF], F32, tag="agg32", bufs=3,
                                  name="agg32")
                nc.scalar.activation(out=agg32[:], in_=agg_ps[:], func=AF.Copy)
                # batch-pair transposes; aggT[:, j, :] holds batches 2j
                # (partitions 0:64) and 2j+1 (64:128), fp16-cast on evacuation
                aggT = pool.tile([128, 4, 128], F16, tag="aggT", bufs=6,
                                 name="aggT")
                for j in range(4):
                    tr_ps = psum.tile([128, 128], F32, space="PSUM",
                                      tag="ps_tr")
                    nc.tensor.transpose(
                        out=tr_ps[:], in_=agg32[:, j * 128:(j + 1) * 128],
                        identity=ident32[:])
                    nc.scalar.activation(out=aggT[:, j, :], in_=tr_ps[:],
                                         func=AF.Copy)
                # h2 = relu(agg_gated @ Wg + bg), 4 batches per PSUM bank via
                # block-diagonal pair weights
                sums = pool.tile([128, 2], F32, tag="sums", bufs=4, name="sums")
                sqs = pool.tile([128, 2], F32, tag="sqs", bufs=4, name="sqs")
                h2g = []
                for g in range(2):
                    h2_ps = psum.tile([128, BF], F32, space="PSUM",
                                      tag="ps_h2", bufs=3)
                    nc.tensor.matmul(out=h2_ps[:], lhsT=ones16[:],
                                     rhs=bg4_sb[:], start=True, stop=False)
                    for jj in range(2):
                        j = g * 2 + jj
                        nc.tensor.matmul(
                            out=h2_ps[:, jj * 2 * H:(jj + 1) * 2 * H],
                            lhsT=aggT[:, j, :],
                            rhs=wgebz[j][:],
                            start=False, stop=(jj == 1))
                    h2 = pool.tile([128, BF], F16, tag=f"h2_{g}", bufs=3,
                                   name=f"h2_{g}")
                    nc.scalar.activation(out=h2[:], in_=h2_ps[:], func=AF.Relu,
                                         accum_out=sums[:, g:g + 1])
                    sqscr = pool.tile([128, BF], F16, tag="sqscr", bufs=3,
                                      name="sqscr")
                    nc.vector.tensor_tensor(out=sqscr[:], in0=h2[:],
                                            in1=h2[:], op=ALU.mult)
                    nc.vector.tensor_reduce(out=sqs[:, g:g + 1],
                                            in_=sqscr[:], axis=AX.X,
                                            op=ALU.add)
                    h2g.append(h2)
                # BN1 per-node affine coefficients (DVE column math)
                rsumt = pool.tile([128, 1], F32, tag="rsumt", bufs=4, name="rsumt")
                nc.vector.tensor_reduce(out=rsumt[:], in_=sums[:], axis=AX.X,
                                        op=ALU.add)
                sqsumt = pool.tile([128, 1], F32, tag="sqsumt", bufs=4, name="sqsumt")
                nc.vector.tensor_reduce(out=sqsumt[:], in_=sqs[:], axis=AX.X,
                                        op=ALU.add)
                mean = pool.tile([128, 1], F32, tag="mean", bufs=4, name="mean")
                nc.vector.tensor_scalar_mul(mean[:], rsumt[:], 1.0 / (B * H))
                msqe = pool.tile([128, 1], F32, tag="msqe", bufs=4, name="msqe")
                nc.vector.tensor_scalar_mul(msqe[:], sqsumt[:], 1.0 / (B * H))
                nc.vector.tensor_scalar_add(msqe[:], msqe[:], BN_EPS)
                var = pool.tile([128, 1], F32, tag="var", bufs=4, name="var")
                nc.vector.tensor_tensor(out=var[:], in0=mean[:], in1=mean[:],
                                        op=ALU.mult)
                nc.vector.tensor_tensor(out=var[:], in0=msqe[:], in1=var[:],
                                        op=ALU.subtract)
                inv = pool.tile([128, 1], F32, tag="inv", bufs=4, name="inv")
                nc.vector.reciprocal(out=inv[:], in_=var[:])
                rstd = pool.tile([128, 1], F32, tag="rstd", bufs=4, name="rstd")
                nc.scalar.sqrt(out=rstd[:], in_=inv[:])
                aco = pool.tile([128, 1], F32, tag="aco", bufs=4, name="aco")
                nc.vector.tensor_tensor(out=aco[:], in0=rstd[:],
                                        in1=bn1all[:, t, 0:1], op=ALU.mult)
                bco = pool.tile([128, 1], F32, tag="bco", bufs=4, name="bco")
                nc.vector.tensor_tensor(out=bco[:], in0=mean[:], in1=aco[:],
                                        op=ALU.mult)
                nc.vector.tensor_tensor(out=bco[:], in0=bn1all[:, t, 1:2],
                                        in1=bco[:], op=ALU.subtract)
                # apply BN1 (ACT: out = aco*h2 + bco) and fold into pool max
                for g in range(2):
                    if t == 0:
                        nc.vector.tensor_scalar(
                            out=poolacc[g][:], in0=h2g[g][:],
                            scalar1=aco[:, 0:1], scalar2=bco[:, 0:1],
                            op0=ALU.mult, op1=ALU.add)
                    else:
                        h2n = pool.tile([128, BF], F16, tag="h2n", bufs=3,
                                        name="h2n")
                        nc.vector.tensor_scalar(
                            out=h2n[:], in0=h2g[g][:],
                            scalar1=aco[:, 0:1], scalar2=bco[:, 0:1],
                            op0=ALU.mult, op1=ALU.add)
                        nc.vector.tensor_tensor(out=poolacc[g][:],
                                                in0=poolacc[g][:], in1=h2n[:],
                                                op=ALU.max)

            # fold pooled partials: per batch, transpose + reduce over nodes
            pacc32 = [cpool.tile([128, BF], F32, tag=f"pacc32_{g}",
                                 name=f"pacc32_{g}") for g in range(2)]
            for g in range(2):
                nc.scalar.activation(out=pacc32[g][:], in_=poolacc[g][:],
                                     func=AF.Copy)
            for b in range(B):
                g, j = b // 4, b % 4
                hT_ps = psum.tile([128, 128], F32, space="PSUM", tag="ps_tr")
                nc.tensor.transpose(
                    out=hT_ps[:], in_=pacc32[g][:, j * H:(j + 1) * H],
                    identity=ident32[:])
                nc.vector.tensor_reduce(out=pooled[:, b:b + 1], in_=hT_ps[:],
                                        axis=AX.X, op=ALU.max)

            # ---- phase C: AllGather + local max + replicated head ----
            if skip_collective:
                pooledf = pooled
            else:
                p_in = dpool.tile([H, B], F32)
                p_out = dpool.tile([NCORES, H, B], F32)
                nc.gpsimd.dma_start(out=p_in[:], in_=pooled[:])
                nc.gpsimd.collective_compute(
                    "AllGather", ALU.bypass,
                    replica_groups=[list(range(NCORES))],
                    ins=[p_in.opt()], outs=[p_out.opt()],
                )
                pg = cpool.tile([H, NCORES, B], F32)
                nc.sync.dma_start(out=pg[:],
                                  in_=p_out[:].rearrange("c p b -> p c b"))
                pooledf = cpool.tile([H, B], F32)
                nc.vector.tensor_reduce(
                    out=pooledf[:], in_=pg[:].rearrange("p c b -> p b c"),
                    axis=AX.X, op=ALU.max)

            def bn_free8(z, nrows, gamma_col, beta_col, tag):
                """BN over the 8 free-dim entries of z [nrows, 8] -> new tile."""
                rs = pool.tile([nrows, 1], F32, tag=f"{tag}_rs")
                nc.vector.tensor_reduce(out=rs[:], in_=z[:], axis=AX.X,
                                        op=ALU.add)
                nc.vector.tensor_scalar_mul(rs[:], rs[:], 1.0 / B)
                sqt = pool.tile([nrows, B], F32, tag=f"{tag}_sqt")
                sq = pool.tile([nrows, 1], F32, tag=f"{tag}_sq")
                nc.scalar.activation(out=sqt[:], in_=z[:], func=AF.Square,
                                     accum_out=sq[:])
                nc.vector.tensor_scalar_mul(sq[:], sq[:], 1.0 / B)
                v = pool.tile([nrows, 1], F32, tag=f"{tag}_v")
                nc.vector.tensor_tensor(out=v[:], in0=rs[:], in1=rs[:],
                                        op=ALU.mult)
                nc.vector.tensor_tensor(out=v[:], in0=sq[:], in1=v[:],
                                        op=ALU.subtract)
                nc.vector.tensor_scalar_add(v[:], v[:], BN_EPS)
                iv = pool.tile([nrows, 1], F32, tag=f"{tag}_iv")
                nc.vector.reciprocal(out=iv[:], in_=v[:])
                rst = pool.tile([nrows, 1], F32, tag=f"{tag}_rst")
                nc.scalar.sqrt(out=rst[:], in_=iv[:])
                ac = pool.tile([nrows, 1], F32, tag=f"{tag}_ac")
                nc.vector.tensor_tensor(out=ac[:], in0=rst[:], in1=gamma_col,
                                        op=ALU.mult)
                bc = pool.tile([nrows, 1], F32, tag=f"{tag}_bc")
                nc.vector.tensor_tensor(out=bc[:], in0=rs[:], in1=ac[:],
                                        op=ALU.mult)
                nc.vector.tensor_tensor(out=bc[:], in0=beta_col, in1=bc[:],
                                        op=ALU.subtract)
                zn = pool.tile([nrows, B], F32, tag=f"{tag}_zn")
                nc.vector.tensor_scalar(out=zn[:], in0=z[:],
                                        scalar1=ac[:, 0:1], scalar2=bc[:, 0:1],
                                        op0=ALU.mult, op1=ALU.add)
                return zn

            # head in feature-major layout: features on partitions, batch on
            # free. Biases ride ACT's per-partition bias port; no transposes.
            z1n = []
            for j in range(2):
                z1_ps = psum.tile([H, B], F32, space="PSUM", tag="ps_b", bufs=1)
                nc.tensor.matmul(out=z1_ps[:],
                                 lhsT=wf1_sb[:, j * 128:(j + 1) * 128],
                                 rhs=pooledf[:], start=True, stop=True)
                z1T = pool.tile([H, B], F32, tag=f"z1T_{j}")
                nc.scalar.activation(out=z1T[:], in_=z1_ps[:], func=AF.Relu,
                                     bias=bf1c_sb[:, j:j + 1])
                z1n.append(bn_free8(z1T, 128, g2_sb[:, j:j + 1],
                                    be2_sb[:, j:j + 1], f"bn2_{j}"))
            z2_ps = psum.tile([FC2, B], F32, space="PSUM", tag="ps_b", bufs=1)
            nc.tensor.matmul(out=z2_ps[:], lhsT=wf2a_sb[:], rhs=z1n[0][:],
                             start=True, stop=False)
            nc.tensor.matmul(out=z2_ps[:], lhsT=wf2b_sb[:], rhs=z1n[1][:],
                             start=False, stop=True)
            z2T = pool.tile([FC2, B], F32, tag="z2T")
            nc.scalar.activation(out=z2T[:], in_=z2_ps[:], func=AF.Relu,
                                 bias=bf2c_sb[:])
            z2n = bn_free8(z2T, FC2, g3_sb[:, 0:1], be3_sb[:, 0:1], "bn3")
            # logits^T [OUT, B], add bias, transpose to [B, OUT], softmax
            lg_ps = psum.tile([OUT, B], F32, space="PSUM", tag="ps_b", bufs=1)
            nc.tensor.matmul(out=lg_ps[:], lhsT=wo_sb[:], rhs=z2n[:],
                             start=True, stop=True)
            lgT = pool.tile([OUT, B], F32, tag="lgT")
            nc.vector.tensor_scalar(out=lgT[:], in0=lg_ps[:],
                                    scalar1=boc_sb[:, 0:1], scalar2=None,
                                    op0=ALU.add)
            lgt_ps = psum.tile([B, OUT], F32, space="PSUM", tag="ps_b", bufs=1)
            nc.tensor.transpose(out=lgt_ps[:], in_=lgT[:],
                                identity=ident32[0:OUT, 0:OUT])
            lg = pool.tile([B, OUT], F32, tag="lg")
            nc.vector.tensor_copy(out=lg[:], in_=lgt_ps[:])
            mx = pool.tile([B, 1], F32, tag="mx")
            nc.vector.tensor_reduce(out=mx[:], in_=lg[:], axis=AX.X, op=ALU.max)
            ex = pool.tile([B, OUT], F32, tag="ex")
            nc.vector.tensor_scalar(out=ex[:], in0=lg[:], scalar1=mx[:, 0:1],
                                    scalar2=None, op0=ALU.subtract)
            nc.scalar.activation(out=ex[:], in_=ex[:], func=AF.Exp)
            ssum = pool.tile([B, 1], F32, tag="ssum")
            nc.vector.tensor_reduce(out=ssum[:], in_=ex[:], axis=AX.X,
                                    op=ALU.add)
            sinv = pool.tile([B, 1], F32, tag="sinv")
            nc.vector.reciprocal(out=sinv[:], in_=ssum[:])
            sm = pool.tile([B, OUT], F32, tag="sm")
            nc.vector.tensor_scalar(out=sm[:], in0=ex[:], scalar1=sinv[:, 0:1],
                                    scalar2=None, op0=ALU.mult)
            nc.sync.dma_start(out=out_t[:], in_=sm[:])
    nc.compile()
    return nc


def preprocess(x, src, dst, edge_w):
    """Host marshalling: node-major x table + sorted/padded edge tiles."""
    order = np.argsort(dst, kind="stable")
    ss = src[order].astype(np.int64)
    ds = dst[order].astype(np.int64)
    ws = edge_w[order].astype(np.float32)
    tile_id = ds // 128
    dloc = ds % 128
    counts = np.bincount(tile_id, minlength=NTILE)
    cpt = int(np.ceil(counts.max() / 128))
    slots = cpt * 128

    gidx_all = np.zeros((NTILE, slots), np.int16)
    sval = np.zeros((NTILE, slots), np.float32)
    sloc = np.zeros((NTILE, slots), np.int64)
    offs = np.concatenate([[0], np.cumsum(counts)])
    for t in range(NTILE):
        cnt = counts[t]
        seg = slice(offs[t], offs[t + 1])
        # order each tile's edges by ascending src so gather descriptors
        # read ascending HBM addresses (DRAM row-buffer locality); the
        # segment-sum is order-invariant since S follows the slot order
        o = np.argsort(ss[seg], kind="stable")
        gidx_all[t, :cnt] = ss[seg][o]
        sval[t, :cnt] = ws[seg][o]
        sloc[t, :cnt] = dloc[seg][o]

    # per-chunk (dloc, w) columns; padded slots get dloc=200 (unmatchable)
    dlocw = np.zeros((NTILE, 128, 2 * cpt), np.float32)
    for t in range(NTILE):
        dl = sloc[t].reshape(cpt, 128).T.astype(np.float32)
        wv = sval[t].reshape(cpt, 128).T
        dl[wv == 0.0] = 200.0
        dlocw[t, :, 0::2] = dl
        dlocw[t, :, 1::2] = wv
    dlocw = dlocw.astype(np.float32)

    # wrapped int16 index tables: [16, slots//16] replicated to 128 partitions
    gidx_w = np.zeros((NTILE, 128, slots // 16), np.int16)
    for t in range(NTILE):
        base = gidx_all[t].reshape(slots // 16, 16).T
        gidx_w[t] = np.tile(base, (8, 1))

    # per-core tile order (descending edge count) and per-slot chunk counts
    order_pc = np.zeros((NCORES, TPC), np.int64)
    for c in range(NCORES):
        tl = np.arange(c * TPC, (c + 1) * TPC)
        order_pc[c] = tl[np.argsort(-counts[tl], kind="stable")]
    cpts = tuple(
        int(np.ceil(max(counts[order_pc[c][i]] for c in range(NCORES)) / 128))
        for i in range(TPC)
    )
    xt = np.ascontiguousarray(
        np.asarray(x, np.float32).transpose(1, 0, 2).reshape(N, BF)
    ).astype(f16)
    return xt, gidx_w, dlocw, cpts, order_pc


def make_in_maps(inputs, xt, gidx_w, dlocw, cpts, order_pc):
    g1 = np.asarray(inputs["g1"], np.float32).reshape(NTILE, 128)
    beta1 = np.asarray(inputs["beta1"], np.float32).reshape(NTILE, 128)
    bn1 = np.stack([g1, beta1], axis=-1)  # [NTILE, 128, 2]

    f32 = lambda a: np.ascontiguousarray(np.asarray(a, np.float32))
    wp = np.zeros((128, 849), np.float32)
    wp[:, 0:256] = f32(inputs["Wf1"])
    wf2 = f32(inputs["Wf2"]).reshape(2, H, FC2)
    wp[:, 256:384] = wf2[0]
    wp[:, 384:512] = wf2[1]
    wp[:, 512:640] = np.tile(f32(inputs["Wg"]), (2, 1))
    wp[0:SE_D, 640:768] = np.concatenate([f32(inputs["Wop"])] * 2, axis=1)
    wp[0:F, 768:800] = f32(inputs["W1"])
    wp[0:SE_D, 800:832] = f32(inputs["W2"])
    wp[:, 832:836] = f32(inputs["Wo"])
    wp[0:SE_D, 836] = f32(inputs["b1"])
    wp[0:SE_D, 837] = f32(inputs["b2"])
    wp[:, 838] = np.tile(f32(inputs["bop"]), 2)
    wp[:, 839:841] = f32(inputs["bf1"]).reshape(2, H).T
    wp[:, 841] = f32(inputs["bf2"])
    wp[:, 842:844] = f32(inputs["g2"]).reshape(2, H).T
    wp[:, 844:846] = f32(inputs["beta2"]).reshape(2, H).T
    wp[:, 846] = f32(inputs["g3"])
    wp[:, 847] = f32(inputs["beta3"])
    wp[0:OUT, 848] = f32(inputs["bo"])
    shared = {
        "xt": xt,
        "iota16": np.tile(np.arange(128, dtype=np.float32), (128, 1)).astype(f16),
        "wpack": wp,
        "bg4": np.tile(f32(inputs["bg"]).reshape(1, H), (1, 4)).astype(f16),
    }
    in_maps = []
    for c in range(NCORES):
        order = order_pc[c]
        m = dict(shared)
        m["xs"] = np.ascontiguousarray(xt[c * (N // NCORES):(c + 1) * (N // NCORES)])
        m["gidx"] = np.ascontiguousarray(np.concatenate(
            [gidx_w[gt][:, :cpts[i] * 8] for i, gt in enumerate(order)], axis=1))
        m["dlw"] = np.ascontiguousarray(np.concatenate(
            [dlocw[gt][:, :2 * cpts[i]] for i, gt in enumerate(order)], axis=1))
        m["bn1p"] = np.ascontiguousarray(bn1[order])
        in_maps.append(m)
    return in_maps


_CACHE = {}
LAST_RESULT = None  # BassKernelResults of the most recent kernel() call


def kernel(**inputs):
    global LAST_RESULT
    xt, gidx_w, dlocw, cpts, order_pc = preprocess(
        np.asarray(inputs["x"]), np.asarray(inputs["src"]),
        np.asarray(inputs["dst"]), np.asarray(inputs["edge_w"]))
    if cpts not in _CACHE:
        _CACHE[cpts] = build_kernel(cpts)
    nc = _CACHE[cpts]
    in_maps = make_in_maps(inputs, xt, gidx_w, dlocw, cpts, order_pc)
    trace = os.environ.get("BASS_KERNEL_TRACE", "0") == "1"
    res = run_bass_kernel_spmd(nc, in_maps, list(range(NCORES)), trace=trace)
    LAST_RESULT = res
    return np.asarray(res.results[0]["out"], np.float32)


# revision 59
# speedup vs baseline: 1.0417x; 1.0275x over previous
"""Trainium2 Bass kernel for nn_BaseGCNModel_addSE (gnn_message_passing).

SPMD over 8 NeuronCores, data laid out so the SE gate commutes with the
sparse aggregation:

    agg = A @ (x * (1+gate)) = (A @ x) * (1+gate)

since the gate is constant along the contracted node axis. The kernel
gathers messages directly from the host-marshalled node-major table
xt [N, B*F] (fp16, 1KB rows), segment-sums them on the PE via streamed
fp16 one-hot blocks (edge weights folded in), and applies the gate by
scaling Wg per batch. Every core owns 16 of the 128 dst-node tiles; BN1
is node-local; pooled partials are combined with AllGather + local max
(cheaper than AllReduce in the fabric); the FC head runs replicated in
feature-major layout (no transposes, per-partition bias/scale on ACT).

Mid-pipeline runs in fp16: agg is evacuated to fp16, transposed in
128x128 pairs on the PE (fp16 identity, 1 cyc/row), and the gate-scaled
Wg matmuls run fp16 (4x cheaper than the fp32 path).
"""

import os
import sys

for _p in ("/opt/trn_rl_repo", "/root/.axon_site/_ro/trn_rl_repo"):
    if _p not in sys.path:
        sys.path.insert(0, _p)

import numpy as np

import concourse.bass as bass
import concourse.bacc as bacc
import concourse.mybir as mybir
import concourse.tile as tile
from concourse.bass_utils import run_bass_kernel_spmd
from concourse.masks import make_identity

f16 = np.float16
F32 = mybir.dt.float32
F16 = mybir.dt.float16
I16 = mybir.dt.int16
AF = mybir.ActivationFunctionType
ALU = mybir.AluOpType
AX = mybir.AxisListType

B, N, F, E, H = 8, 16384, 64, 262144, 128
SE_D = 32
FC1, FC2, OUT = 256, 128, 4
BN_EPS = 1e-3
NCORES = 8
NTILE = 128            # global 128-node dst tiles
TPC = NTILE // NCORES  # dst tiles per core (16)
BF = B * F             # 512, xt row width
MAX_GATHER = 512      # SWDGE ring limit: >1024 descriptors per gather crashes


def build_kernel(cpts, skip_collective: bool = False, phases: str = "GB"):
    """Build the SPMD program. cpts[i] = chunks (of 128 edges) for tile slot i
    (per-core tiles are sorted by descending edge count, so slot i's static
    size is the max of the i-th order statistic across cores)."""
    if isinstance(cpts, int):
        cpts = (cpts,) * TPC
    slots_i = [c * 128 for c in cpts]
    total_slots = sum(slots_i)
    offs_i = np.concatenate([[0], np.cumsum(slots_i)]).astype(int)
    nc = bacc.Bacc("TRN2", target_bir_lowering=False, debug=False,
                   num_devices=NCORES)

    # inputs (identical content on every core unless noted "per-core")
    xt = nc.dram_tensor("xt", [N, BF], F16, kind="ExternalInput")
    xs = nc.dram_tensor("xs", [N // NCORES, BF], F16, kind="ExternalInput")  # per-core x slice
    gidx = nc.dram_tensor("gidx", [128, total_slots // 16], I16, kind="ExternalInput")  # per-core
    # per-chunk (dloc, w) column pairs; S one-hot blocks are built on-chip
    dlw = nc.dram_tensor("dlw", [128, 2 * (total_slots // 128)], F32,
                         kind="ExternalInput")  # per-core
    iota16 = nc.dram_tensor("iota16", [128, 128], F16, kind="ExternalInput")
    bn1p = nc.dram_tensor("bn1p", [TPC, 128, 2], F32, kind="ExternalInput")             # per-core
    wpack = nc.dram_tensor("wpack", [128, 849], F32, kind="ExternalInput")
    bg4 = nc.dram_tensor("bg4", [1, BF], F16, kind="ExternalInput")  # bg tiled 4x

    out_t = nc.dram_tensor("out", [B, OUT], F32, kind="ExternalOutput")

    with tile.TileContext(nc) as tc:
        with (
            tc.tile_pool(name="const", bufs=1) as cpool,
            tc.tile_pool(name="sbuf", bufs=2) as pool,
            tc.tile_pool(name="psum", bufs=2, space="PSUM") as psum,
            tc.tile_pool(name="dram", bufs=1, space="DRAM") as dpool,
        ):
            # ---- constants / weights ----
            ident32 = cpool.tile([128, 128], F32)
            make_identity(nc, ident32[:])
            ident16 = cpool.tile([128, 128], F16)
            make_identity(nc, ident16[:])
            ones16 = cpool.tile([1, 128], F16)
            nc.vector.memset(ones16[:], 1.0)

            wpack_sb = cpool.tile([128, 849], F32)
            nc.sync.dma_start(out=wpack_sb[:], in_=wpack[:])
            bg4_sb = cpool.tile([1, BF], F16)
            nc.sync.dma_start(out=bg4_sb[:], in_=bg4[:])
            dlw_sb = cpool.tile([128, 2 * (total_slots // 128)], F32)
            nc.sync.dma_start(out=dlw_sb[:], in_=dlw[:])
            gidx_all = cpool.tile([128, total_slots // 16], I16)
            nc.sync.dma_start(out=gidx_all[:], in_=gidx[:])
            bn1all = cpool.tile([128, TPC, 2], F32)
            nc.sync.dma_start(out=bn1all[:],
                              in_=bn1p[:].rearrange("t p c -> p t c"))
            iota_sb = cpool.tile([128, 128], F16)
            nc.sync.dma_start(out=iota_sb[:], in_=iota16[:])
            wf1_sb = wpack_sb[:, 0:256]
            wf2a_sb = wpack_sb[:, 256:384]
            wf2b_sb = wpack_sb[:, 384:512]
            wg2_sb = wpack_sb[:, 512:640]
            wop2_sb = wpack_sb[0:SE_D, 640:768]
            w1_sb = wpack_sb[0:F, 768:800]
            w2_sb = wpack_sb[0:SE_D, 800:832]
            wo_sb = wpack_sb[:, 832:836]
            b1_sb = wpack_sb[0:SE_D, 836:837]
            b2_sb = wpack_sb[0:SE_D, 837:838]
            bop2_sb = wpack_sb[:, 838:839]
            bf1c_sb = wpack_sb[:, 839:841]
            bf2c_sb = wpack_sb[:, 841:842]
            g2_sb = wpack_sb[:, 842:844]
            be2_sb = wpack_sb[:, 844:846]
            g3_sb = wpack_sb[:, 846:847]
            be3_sb = wpack_sb[:, 847:848]
            boc_sb = wpack_sb[0:OUT, 848:849]

            # ---- phase G: SE gate (max-pool over nodes + tiny MLP) ----
            gates = None
            if "G" in phases:
                # sharded x-scan: each core reduces its N/8 slice, then
                # AllGather + local max of the [F, B] partials
                rows = N // NCORES
                q = rows // 4
                xs_sb = [cpool.tile([128, q // 128, BF], F16,
                                    tag=f"xs_sb{i}", name=f"xs_sb{i}")
                         for i in range(4)]
                reds = [cpool.tile([128, BF], F32, tag=f"reds{i}",
                                   name=f"reds{i}") for i in range(4)]
                for i in range(4):
                    nc.sync.dma_start(
                        out=xs_sb[i][:],
                        in_=xs[i * q:(i + 1) * q].rearrange(
                            "(p c) w -> p c w", p=128))
                    nc.vector.tensor_reduce(
                        out=reds[i][:],
                        in_=xs_sb[i][:].rearrange("p c w -> p w c"),
                        axis=AX.X, op=ALU.max,
                    )
                nc.vector.tensor_tensor(out=reds[0][:], in0=reds[0][:],
                                        in1=reds[1][:], op=ALU.max)
                nc.vector.tensor_tensor(out=reds[2][:], in0=reds[2][:],
                                        in1=reds[3][:], op=ALU.max)
                redpart = cpool.tile([128, BF], F32)
                nc.vector.tensor_tensor(out=redpart[:], in0=reds[0][:],
                                        in1=reds[2][:], op=ALU.max)
                pp = cpool.tile([F, B], F32)
                for b in range(B):
                    red_ps = psum.tile([F, 128], F32, space="PSUM",
                                       tag="ps_tr", bufs=3)
                    nc.tensor.transpose(
                        out=red_ps[:], in_=redpart[:, b * F:(b + 1) * F],
                        identity=ident32[:])
                    nc.vector.tensor_reduce(out=pp[:, b:b + 1], in_=red_ps[:],
                                            axis=AX.X, op=ALU.max)
                if skip_collective:
                    ppf = pp
                else:
                    r_in = dpool.tile([F, B], F32)
                    r_out = dpool.tile([NCORES, F, B], F32)
                    nc.gpsimd.dma_start(out=r_in[:], in_=pp[:])
                    nc.gpsimd.collective_compute(
                        "AllGather", ALU.bypass,
                        replica_groups=[list(range(NCORES))],
                        ins=[r_in.opt()], outs=[r_out.opt()],
                    )
                    ppg = cpool.tile([F, NCORES, B], F32)
                    nc.sync.dma_start(out=ppg[:],
                                      in_=r_out[:].rearrange("c p b -> p c b"))
                    ppf = cpool.tile([F, B], F32)
                    nc.vector.tensor_reduce(
                        out=ppf[:], in_=ppg[:].rearrange("p c b -> p b c"),
                        axis=AX.X, op=ALU.max)
                # gate MLP, all batches at once
                a1_ps = psum.tile([SE_D, B], F32, space="PSUM", tag="ps_b", bufs=1)
                nc.tensor.matmul(out=a1_ps[:], lhsT=w1_sb[:], rhs=ppf[:],
                                 start=True, stop=True)
                a1 = pool.tile([SE_D, B], F32, tag="a1")
                nc.scalar.activation(out=a1[:], in_=a1_ps[:], func=AF.Relu,
                                     bias=b1_sb[:])
                a2_ps = psum.tile([SE_D, B], F32, space="PSUM", tag="ps_b", bufs=1)
                nc.tensor.matmul(out=a2_ps[:], lhsT=w2_sb[:], rhs=a1[:],
                                 start=True, stop=True)
                a2 = pool.tile([SE_D, B], F32, tag="a2")
                nc.scalar.activation(out=a2[:], in_=a2_ps[:], func=AF.Relu,
                                     bias=b2_sb[:])
                g_ps = psum.tile([2 * F, B], F32, space="PSUM", tag="ps_b", bufs=1)
                nc.tensor.matmul(out=g_ps[:], lhsT=wop2_sb[:], rhs=a2[:],
                                 start=True, stop=True)
                gates = cpool.tile([2 * F, B], F32)
                nc.scalar.activation(out=gates[:], in_=g_ps[:],
                                     func=AF.Sigmoid, bias=bop2_sb[:])
                nc.vector.tensor_scalar_add(gates[:], gates[:], 1.0)

            # block-diagonal gate-scaled Wg pairs (fp16): for batch pair
            # (2j, 2j+1), rows 0:64 x cols 0:128 hold Wg*gate[2j] and rows
            # 64:128 x cols 128:256 hold Wg*gate[2j+1]. One h2 matmul then
            # computes both batches (contraction over the stacked f axis).
            wgebz = []
            for j in range(4):
                wz = cpool.tile([2 * F, 2 * H], F16, tag=f"wgebz_{j}",
                                name=f"wgebz_{j}")
                nc.vector.memset(wz[:], 0.0)
                if gates is not None:
                    nc.vector.tensor_scalar(
                        out=wz[0:F, 0:H], in0=wg2_sb[0:F, :],
                        scalar1=gates[0:F, 2 * j:2 * j + 1],
                        scalar2=None, op0=ALU.mult)
                    nc.vector.tensor_scalar(
                        out=wz[F:2 * F, H:2 * H], in0=wg2_sb[F:2 * F, :],
                        scalar1=gates[F:2 * F, 2 * j + 1:2 * j + 2],
                        scalar2=None, op0=ALU.mult)
                else:
                    nc.vector.tensor_copy(out=wz[0:F, 0:H],
                                          in_=wg2_sb[0:F, :])
                    nc.vector.tensor_copy(out=wz[F:2 * F, H:2 * H],
                                          in_=wg2_sb[F:2 * F, :])
                wgebz.append(wz)

            # ---- phase B: SpMM + Wg + BN1 + pool partial ----
            pooled = cpool.tile([H, B], F32)
            poolacc = [cpool.tile([128, BF], F16, tag=f"poolacc_{g}",
                                  name=f"poolacc_{g}")
                       for g in range(2)]

            for t in range(TPC) if "B" in phases else []:
                cpt_t = cpts[t]
                slots_t = slots_i[t]
                off_t = int(offs_i[t])

                # build the one-hot S blocks on-chip: row e of chunk k gets
                # w at column dloc ((iota == dloc) * w), zeros elsewhere
                s_sb = pool.tile([128, slots_t], F16, tag="s_sb", bufs=4,
                                 name="s_sb")
                cbase = off_t // 128
                for k in range(cpt_t):
                    c = cbase + k
                    nc.vector.tensor_scalar(
                        out=s_sb[:, k * 128:(k + 1) * 128], in0=iota_sb[:],
                        scalar1=dlw_sb[:, 2 * c:2 * c + 1],
                        scalar2=dlw_sb[:, 2 * c + 1:2 * c + 2],
                        op0=ALU.is_equal, op1=ALU.mult)

                msg = pool.tile([128, cpt_t, BF], F16, tag="msg", bufs=5,
                                name="msg")
                for s0 in range(0, slots_t, MAX_GATHER):
                    n_i = min(MAX_GATHER, slots_t - s0)
                    nc.gpsimd.dma_gather(
                        out_ap=msg[:, s0 // 128:(s0 + n_i) // 128, :],
                        in_ap=xt[:],
                        idxs_ap=gidx_all[:, (off_t + s0) // 16:(off_t + s0 + n_i) // 16],
                        num_idxs=n_i, num_idxs_reg=n_i, elem_size=BF,
                    )
                agg_ps = psum.tile([128, BF], F32, space="PSUM", tag="ps_agg",
                                   bufs=2)
                for k in range(cpt_t):
                    nc.tensor.matmul(
                        out=agg_ps[:],
                        lhsT=s_sb[:, k * 128:(k + 1) * 128],
                        rhs=msg[:, k, :],
                        start=(k == 0), stop=(k == cpt_t - 1),
                    )
                agg32 = pool.tile([128, BF], F32, tag="agg32", bufs=4,
                                  name="agg32")
                nc.scalar.activation(out=agg32[:], in_=agg_ps[:], func=AF.Copy)
                # batch-pair transposes; aggT[:, j, :] holds batches 2j
                # (partitions 0:64) and 2j+1 (64:128), fp16-cast on evacuation
                aggT = pool.tile([128, 4, 128], F16, tag="aggT", bufs=8,
                                 name="aggT")
                for j in range(4):
                    tr_ps = psum.tile([128, 128], F32, space="PSUM",
                                      tag="ps_tr", bufs=3)
                    nc.tensor.transpose(
                        out=tr_ps[:], in_=agg32[:, j * 128:(j + 1) * 128],
                        identity=ident32[:])
                    nc.scalar.activation(out=aggT[:, j, :], in_=tr_ps[:],
                                         func=AF.Copy)
                # h2 = relu(agg_gated @ Wg + bg), 4 batches per PSUM bank via
                # block-diagonal pair weights
                sums = pool.tile([128, 2], F32, tag="sums", bufs=4, name="sums")
                sqs = pool.tile([128, 2], F32, tag="sqs", bufs=4, name="sqs")
                h2g = []
                for g in range(2):
                    h2_ps = psum.tile([128, BF], F32, space="PSUM",
                                      tag="ps_h2", bufs=2)
                    nc.tensor.matmul(out=h2_ps[:], lhsT=ones16[:],
                                     rhs=bg4_sb[:], start=True, stop=False)
                    for jj in range(2):
                        j = g * 2 + jj
                        nc.tensor.matmul(
                            out=h2_ps[:, jj * 2 * H:(jj + 1) * 2 * H],
                            lhsT=aggT[:, j, :],
                            rhs=wgebz[j][:],
                            start=False, stop=(jj == 1))
                    h2 = pool.tile([128, BF], F16, tag=f"h2_{g}", bufs=4,
                                   name=f"h2_{g}")
                    nc.scalar.activation(out=h2[:], in_=h2_ps[:], func=AF.Relu,
                                         accum_out=sums[:, g:g + 1])
                    sqscr = pool.tile([128, BF], F16, tag="sqscr", bufs=3,
                                      name="sqscr")
                    nc.vector.tensor_tensor(out=sqscr[:], in0=h2[:],
                                            in1=h2[:], op=ALU.mult)
                    nc.vector.tensor_reduce(out=sqs[:, g:g + 1],
                                            in_=sqscr[:], axis=AX.X,
                                            op=ALU.add)
                    h2g.append(h2)
                # BN1 per-node affine coefficients (DVE column math)
                rsumt = pool.tile([128, 1], F32, tag="rsumt", bufs=4, name="rsumt")
                nc.vector.tensor_reduce(out=rsumt[:], in_=sums[:], axis=AX.X,
                                        op=ALU.add)
                sqsumt = pool.tile([128, 1], F32, tag="sqsumt", bufs=4, name="sqsumt")
                nc.vector.tensor_reduce(out=sqsumt[:], in_=sqs[:], axis=AX.X,
                                        op=ALU.add)
                mean = pool.tile([128, 1], F32, tag="mean", bufs=4, name="mean")
                nc.vector.tensor_scalar_mul(mean[:], rsumt[:], 1.0 / (B * H))
                msqe = pool.tile([128, 1], F32, tag="msqe", bufs=4, name="msqe")
                nc.vector.tensor_scalar_mul(msqe[:], sqsumt[:], 1.0 / (B * H))
                nc.vector.tensor_scalar_add(msqe[:], msqe[:], BN_EPS)
                var = pool.tile([128, 1], F32, tag="var", bufs=4, name="var")
                nc.vector.tensor_tensor(out=var[:], in0=mean[:], in1=mean[:],
                                        op=ALU.mult)
                nc.vector.tensor_tensor(out=var[:], in0=msqe[:], in1=var[:],
                                        op=ALU.subtract)
                inv = pool.tile([128, 1], F32, tag="inv", bufs=4, name="inv")
                nc.vector.reciprocal(out=inv[:], in_=var[:])
                rstd = pool.tile([128, 1], F32, tag="rstd", bufs=4, name="rstd")
                nc.scalar.sqrt(out=rstd[:], in_=inv[:])
                aco = pool.tile([128, 1], F32, tag="aco", bufs=4, name="aco")
                nc.vector.tensor_tensor(out=aco[:], in0=rstd[:],
                                        in1=bn1all[:, t, 0:1], op=ALU.mult)
                bco = pool.tile([128, 1], F32, tag="bco", bufs=4, name="bco")
                nc.vector.tensor_tensor(out=bco[:], in0=mean[:], in1=aco[:],
                                        op=ALU.mult)
                nc.vector.tensor_tensor(out=bco[:], in0=bn1all[:, t, 1:2],
                                        in1=bco[:], op=ALU.subtract)
                # apply BN1 (ACT: out = aco*h2 + bco) and fold into pool max
                for g in range(2):
                    if t == 0:
                        nc.vector.tensor_scalar(
                            out=poolacc[g][:], in0=h2g[g][:],
                            scalar1=aco[:, 0:1], scalar2=bco[:, 0:1],
                            op0=ALU.mult, op1=ALU.add)
                    else:
                        h2n = pool.tile([128, BF], F16, tag="h2n", bufs=3,
                                        name="h2n")
                        nc.vector.tensor_scalar(
                            out=h2n[:], in0=h2g[g][:],
                            scalar1=aco[:, 0:1], scalar2=bco[:, 0:1],
                            op0=ALU.mult, op1=ALU.add)
                        nc.vector.tensor_tensor(out=poolacc[g][:],
                                                in0=poolacc[g][:], in1=h2n[:],
                                                op=ALU.max)

            # fold pooled partials: per batch, transpose + reduce over nodes
            pacc32 = [cpool.tile([128, BF], F32, tag=f"pacc32_{g}",
                                 name=f"pacc32_{g}") for g in range(2)]
            for g in range(2):
                nc.scalar.activation(out=pacc32[g][:], in_=poolacc[g][:],
                                     func=AF.Copy)
            for b in range(B):
                g, j = b // 4, b % 4
                hT_ps = psum.tile([128, 128], F32, space="PSUM", tag="ps_tr", bufs=3)
                nc.tensor.transpose(
                    out=hT_ps[:], in_=pacc32[g][:, j * H:(j + 1) * H],
                    identity=ident32[:])
                nc.vector.tensor_reduce(out=pooled[:, b:b + 1], in_=hT_ps[:],
                                        axis=AX.X, op=ALU.max)

            # ---- phase C: AllGather + local max + replicated head ----
            if skip_collective:
                pooledf = pooled
            else:
                p_in = dpool.tile([H, B], F32)
                p_out = dpool.tile([NCORES, H, B], F32)
                nc.gpsimd.dma_start(out=p_in[:], in_=pooled[:])
                nc.gpsimd.collective_compute(
                    "AllGather", ALU.bypass,
                    replica_groups=[list(range(NCORES))],
                    ins=[p_in.opt()], outs=[p_out.opt()],
                )
                pg = cpool.tile([H, NCORES, B], F32)
                nc.sync.dma_start(out=pg[:],
                                  in_=p_out[:].rearrange("c p b -> p c b"))
                pooledf = cpool.tile([H, B], F32)
                nc.vector.tensor_reduce(
                    out=pooledf[:], in_=pg[:].rearrange("p c b -> p b c"),
                    axis=AX.X, op=ALU.max)

            def bn_free8(z, nrows, gamma_col, beta_col, tag):
                """BN over the 8 free-dim entries of z [nrows, 8] -> new tile."""
                rs = pool.tile([nrows, 1], F32, tag=f"{tag}_rs")
                nc.vector.tensor_reduce(out=rs[:], in_=z[:], axis=AX.X,
                                        op=ALU.add)
                nc.vector.tensor_scalar_mul(rs[:], rs[:], 1.0 / B)
                sqt = pool.tile([nrows, B], F32, tag=f"{tag}_sqt")
                sq = pool.tile([nrows, 1], F32, tag=f"{tag}_sq")
                nc.scalar.activation(out=sqt[:], in_=z[:], func=AF.Square,
                                     accum_out=sq[:])
                nc.vector.tensor_scalar_mul(sq[:], sq[:], 1.0 / B)
                v = pool.tile([nrows, 1], F32, tag=f"{tag}_v")
                nc.vector.tensor_tensor(out=v[:], in0=rs[:], in1=rs[:],
                                        op=ALU.mult)
                nc.vector.tensor_tensor(out=v[:], in0=sq[:], in1=v[:],
                                        op=ALU.subtract)
                nc.vector.tensor_scalar_add(v[:], v[:], BN_EPS)
                iv = pool.tile([nrows, 1], F32, tag=f"{tag}_iv")
                nc.vector.reciprocal(out=iv[:], in_=v[:])
                rst = pool.tile([nrows, 1], F32, tag=f"{tag}_rst")
                nc.scalar.sqrt(out=rst[:], in_=iv[:])
                ac = pool.tile([nrows, 1], F32, tag=f"{tag}_ac")
                nc.vector.tensor_tensor(out=ac[:], in0=rst[:], in1=gamma_col,
                                        op=ALU.mult)
                bc = pool.tile([nrows, 1], F32, tag=f"{tag}_bc")
                nc.vector.tensor_tensor(out=bc[:], in0=rs[:], in1=ac[:],
                                        op=ALU.mult)
                nc.vector.tensor_tensor(out=bc[:], in0=beta_col, in1=bc[:],
                                        op=ALU.subtract)
                zn = pool.tile([nrows, B], F32, tag=f"{tag}_zn")
                nc.vector.tensor_scalar(out=zn[:], in0=z[:],
                                        scalar1=ac[:, 0:1], scalar2=bc[:, 0:1],
                                        op0=ALU.mult, op1=ALU.add)
                return zn

            # head in feature-major layout: features on partitions, batch on
            # free. Biases ride ACT's per-partition bias port; no transposes.
            z1n = []
            for j in range(2):
                z1_ps = psum.tile([H, B], F32, space="PSUM", tag="ps_b", bufs=1)
                nc.tensor.matmul(out=z1_ps[:],
                                 lhsT=wf1_sb[:, j * 128:(j + 1) * 128],
                                 rhs=pooledf[:], start=True, stop=True)
                z1T = pool.tile([H, B], F32, tag=f"z1T_{j}")
                nc.scalar.activation(out=z1T[:], in_=z1_ps[:], func=AF.Relu,
                                     bias=bf1c_sb[:, j:j + 1])
                z1n.append(bn_free8(z1T, 128, g2_sb[:, j:j + 1],
                                    be2_sb[:, j:j + 1], f"bn2_{j}"))
            z2_ps = psum.tile([FC2, B], F32, space="PSUM", tag="ps_b", bufs=1)
            nc.tensor.matmul(out=z2_ps[:], lhsT=wf2a_sb[:], rhs=z1n[0][:],
                             start=True, stop=False)
            nc.tensor.matmul(out=z2_ps[:], lhsT=wf2b_sb[:], rhs=z1n[1][:],
                             start=False, stop=True)
            z2T = pool.tile([FC2, B], F32, tag="z2T")
            nc.scalar.activation(out=z2T[:], in_=z2_ps[:], func=AF.Relu,
                                 bias=bf2c_sb[:])
            z2n = bn_free8(z2T, FC2, g3_sb[:, 0:1], be3_sb[:, 0:1], "bn3")
            # logits^T [OUT, B], add bias, transpose to [B, OUT], softmax
            lg_ps = psum.tile([OUT, B], F32, space="PSUM", tag="ps_b", bufs=1)
            nc.tensor.matmul(out=lg_ps[:], lhsT=wo_sb[:], rhs=z2n[:],
                             start=True, stop=True)
            lgT = pool.tile([OUT, B], F32, tag="lgT")
            nc.vector.tensor_scalar(out=lgT[:], in0=lg_ps[:],
                                    scalar1=boc_sb[:, 0:1], scalar2=None,
                                    op0=ALU.add)
            lgt_ps = psum.tile([B, OUT], F32, space="PSUM", tag="ps_b", bufs=1)
            nc.tensor.transpose(out=lgt_ps[:], in_=lgT[:],
                                identity=ident32[0:OUT, 0:OUT])
            lg = pool.tile([B, OUT], F32, tag="lg")
            nc.vector.tensor_copy(out=lg[:], in_=lgt_ps[:])
            mx = pool.tile([B, 1], F32, tag="mx")
            nc.vector.tensor_reduce(out=mx[:], in_=lg[:], axis=AX.X, op=ALU.max)
            ex = pool.tile([B, OUT], F32, tag="ex")
            nc.vector.tensor_scalar(out=ex[:], in0=lg[:], scalar1=mx[:, 0:1],
                                    scalar2=None, op0=ALU.subtract)
            nc.scalar.activation(out=ex[:], in_=ex[:], func=AF.Exp)
            ssum = pool.tile([B, 1], F32, tag="ssum")
            nc.vector.tensor_reduce(out=ssum[:], in_=ex[:], axis=AX.X,
                                    op=ALU.add)
            sinv = pool.tile([B, 1], F32, tag="sinv")
            nc.vector.reciprocal(out=sinv[:], in_=ssum[:])
            sm = pool.tile([B, OUT], F32, tag="sm")
            nc.vector.tensor_scalar(out=sm[:], in0=ex[:], scalar1=sinv[:, 0:1],
                                    scalar2=None, op0=ALU.mult)
            nc.sync.dma_start(out=out_t[:], in_=sm[:])
    nc.compile()
    return nc


def preprocess(x, src, dst, edge_w):
    """Host marshalling: node-major x table + sorted/padded edge tiles."""
    order = np.argsort(dst, kind="stable")
    ss = src[order].astype(np.int64)
    ds = dst[order].astype(np.int64)
    ws = edge_w[order].astype(np.float32)
    tile_id = ds // 128
    dloc = ds % 128
    counts = np.bincount(tile_id, minlength=NTILE)
    cpt = int(np.ceil(counts.max() / 128))
    slots = cpt * 128

    gidx_all = np.zeros((NTILE, slots), np.int16)
    sval = np.zeros((NTILE, slots), np.float32)
    sloc = np.zeros((NTILE, slots), np.int64)
    offs = np.concatenate([[0], np.cumsum(counts)])
    for t in range(NTILE):
        cnt = counts[t]
        seg = slice(offs[t], offs[t + 1])
        # order each tile's edges by ascending src so gather descriptors
        # read ascending HBM addresses (DRAM row-buffer locality); the
        # segment-sum is order-invariant since S follows the slot order
        o = np.argsort(ss[seg], kind="stable")
        gidx_all[t, :cnt] = ss[seg][o]
        sval[t, :cnt] = ws[seg][o]
        sloc[t, :cnt] = dloc[seg][o]

    # per-chunk (dloc, w) columns; padded slots get dloc=200 (unmatchable)
    dlocw = np.zeros((NTILE, 128, 2 * cpt), np.float32)
    for t in range(NTILE):
        dl = sloc[t].reshape(cpt, 128).T.astype(np.float32)
        wv = sval[t].reshape(cpt, 128).T
        dl[wv == 0.0] = 200.0
        dlocw[t, :, 0::2] = dl
        dlocw[t, :, 1::2] = wv
    dlocw = dlocw.astype(np.float32)

    # wrapped int16 index tables: [16, slots//16] replicated to 128 partitions
    gidx_w = np.zeros((NTILE, 128, slots // 16), np.int16)
    for t in range(NTILE):
        base = gidx_all[t].reshape(slots // 16, 16).T
        gidx_w[t] = np.tile(base, (8, 1))

    # per-core tile order (descending edge count) and per-slot chunk counts
    order_pc = np.zeros((NCORES, TPC), np.int64)
    for c in range(NCORES):
        tl = np.arange(c * TPC, (c + 1) * TPC)
        order_pc[c] = tl[np.argsort(-counts[tl], kind="stable")]
    cpts = tuple(
        int(np.ceil(max(counts[order_pc[c][i]] for c in range(NCORES)) / 128))
        for i in range(TPC)
    )
    xt = np.ascontiguousarray(
        np.asarray(x, np.float32).transpose(1, 0, 2).reshape(N, BF)
    ).astype(f16)
    return xt, gidx_w, dlocw, cpts, order_pc


def make_in_maps(inputs, xt, gidx_w, dlocw, cpts, order_pc):
    g1 = np.asarray(inputs["g1"], np.float32).reshape(NTILE, 128)
    beta1 = np.asarray(inputs["beta1"], np.float32).reshape(NTILE, 128)
    bn1 = np.stack([g1, beta1], axis=-1)  # [NTILE, 128, 2]

    f32 = lambda a: np.ascontiguousarray(np.asarray(a, np.float32))
    wp = np.zeros((128, 849), np.float32)
    wp[:, 0:256] = f32(inputs["Wf1"])
    wf2 = f32(inputs["Wf2"]).reshape(2, H, FC2)
    wp[:, 256:384] = wf2[0]
    wp[:, 384:512] = wf2[1]
    wp[:, 512:640] = np.tile(f32(inputs["Wg"]), (2, 1))
    wp[0:SE_D, 640:768] = np.concatenate([f32(inputs["Wop"])] * 2, axis=1)
    wp[0:F, 768:800] = f32(inputs["W1"])
    wp[0:SE_D, 800:832] = f32(inputs["W2"])
    wp[:, 832:836] = f32(inputs["Wo"])
    wp[0:SE_D, 836] = f32(inputs["b1"])
    wp[0:SE_D, 837] = f32(inputs["b2"])
    wp[:, 838] = np.tile(f32(inputs["bop"]), 2)
    wp[:, 839:841] = f32(inputs["bf1"]).reshape(2, H).T
    wp[:, 841] = f32(inputs["bf2"])
    wp[:, 842:844] = f32(inputs["g2"]).reshape(2, H).T
    wp[:, 844:846] = f32(inputs["beta2"]).reshape(2, H).T
    wp[:, 846] = f32(inputs["g3"])
    wp[:, 847] = f32(inputs["beta3"])
    wp[0:OUT, 848] = f32(inputs["bo"])
    shared = {
        "xt": xt,
        "iota16": np.tile(np.arange(128, dtype=np.float32), (128, 1)).astype(f16),
        "wpack": wp,
        "bg4": np.tile(f32(inputs["bg"]).reshape(1, H), (1, 4)).astype(f16),
    }
    in_maps = []
    for c in range(NCORES):
        order = order_pc[c]
        m = dict(shared)
        m["xs"] = np.ascontiguousarray(xt[c * (N // NCORES):(c + 1) * (N // NCORES)])
        m["gidx"] = np.ascontiguousarray(np.concatenate(
            [gidx_w[gt][:, :cpts[i] * 8] for i, gt in enumerate(order)], axis=1))
        m["dlw"] = np.ascontiguousarray(np.concatenate(
            [dlocw[gt][:, :2 * cpts[i]] for i, gt in enumerate(order)], axis=1))
        m["bn1p"] = np.ascontiguousarray(bn1[order])
        in_maps.append(m)
    return in_maps


_CACHE = {}
LAST_RESULT = None  # BassKernelResults of the most recent kernel() call


def kernel(**inputs):
    global LAST_RESULT
    xt, gidx_w, dlocw, cpts, order_pc = preprocess(
        np.asarray(inputs["x"]), np.asarray(inputs["src"]),
        np.asarray(inputs["dst"]), np.asarray(inputs["edge_w"]))
    if cpts not in _CACHE:
        _CACHE[cpts] = build_kernel(cpts)
    nc = _CACHE[cpts]
    in_maps = make_in_maps(inputs, xt, gidx_w, dlocw, cpts, order_pc)
    trace = os.environ.get("BASS_KERNEL_TRACE", "0") == "1"
    res = run_bass_kernel_spmd(nc, in_maps, list(range(NCORES)), trace=trace)
    LAST_RESULT = res
    return np.asarray(res.results[0]["out"], np.float32)


# revision 63
# speedup vs baseline: 1.0421x; 1.0004x over previous
"""Trainium2 Bass kernel for nn_BaseGCNModel_addSE (gnn_message_passing).

SPMD over 8 NeuronCores, data laid out so the SE gate commutes with the
sparse aggregation:

    agg = A @ (x * (1+gate)) = (A @ x) * (1+gate)

since the gate is constant along the contracted node axis. The kernel
gathers messages directly from the host-marshalled node-major table
xt [N, B*F] (fp16, 1KB rows), segment-sums them on the PE via streamed
fp16 one-hot blocks (edge weights folded in), and applies the gate by
scaling Wg per batch. Every core owns 16 of the 128 dst-node tiles; BN1
is node-local; pooled partials are combined with AllGather + local max
(cheaper than AllReduce in the fabric); the FC head runs replicated in
feature-major layout (no transposes, per-partition bias/scale on ACT).

Mid-pipeline runs in fp16: agg is evacuated to fp16, transposed in
128x128 pairs on the PE (fp16 identity, 1 cyc/row), and the gate-scaled
Wg matmuls run fp16 (4x cheaper than the fp32 path).
"""

import os
import sys

for _p in ("/opt/trn_rl_repo", "/root/.axon_site/_ro/trn_rl_repo"):
    if _p not in sys.path:
        sys.path.insert(0, _p)

import numpy as np

import concourse.bass as bass
import concourse.bacc as bacc
import concourse.mybir as mybir
import concourse.tile as tile
from concourse.bass_utils import run_bass_kernel_spmd
from concourse.masks import make_identity

f16 = np.float16
F32 = mybir.dt.float32
F16 = mybir.dt.float16
I16 = mybir.dt.int16
AF = mybir.ActivationFunctionType
ALU = mybir.AluOpType
AX = mybir.AxisListType

B, N, F, E, H = 8, 16384, 64, 262144, 128
SE_D = 32
FC1, FC2, OUT = 256, 128, 4
BN_EPS = 1e-3
NCORES = 8
NTILE = 128            # global 128-node dst tiles
TPC = NTILE // NCORES  # dst tiles per core (16)
BF = B * F             # 512, xt row width
MAX_GATHER = 512      # SWDGE ring limit: >1024 descriptors per gather crashes


def build_kernel(cpts, skip_collective: bool = False, phases: str = "GB"):
    """Build the SPMD program. cpts[i] = chunks (of 128 edges) for tile slot i
    (per-core tiles are sorted by descending edge count, so slot i's static
    size is the max of the i-th order statistic across cores)."""
    if isinstance(cpts, int):
        cpts = (cpts,) * TPC
    slots_i = [c * 128 for c in cpts]
    total_slots = sum(slots_i)
    offs_i = np.concatenate([[0], np.cumsum(slots_i)]).astype(int)
    nc = bacc.Bacc("TRN2", target_bir_lowering=False, debug=False,
                   num_devices=NCORES)

    # inputs (identical content on every core unless noted "per-core")
    xt = nc.dram_tensor("xt", [N, BF], F16, kind="ExternalInput")
    xs = nc.dram_tensor("xs", [N // NCORES, BF], F16, kind="ExternalInput")  # per-core x slice
    gidx = nc.dram_tensor("gidx", [128, total_slots // 16], I16, kind="ExternalInput")  # per-core
    # per-chunk (dloc, w) column pairs; S one-hot blocks are built on-chip
    dlw = nc.dram_tensor("dlw", [128, 2 * (total_slots // 128)], F32,
                         kind="ExternalInput")  # per-core
    iota16 = nc.dram_tensor("iota16", [128, 128], F16, kind="ExternalInput")
    bn1p = nc.dram_tensor("bn1p", [TPC, 128, 2], F32, kind="ExternalInput")             # per-core
    wpack = nc.dram_tensor("wpack", [128, 849], F32, kind="ExternalInput")
    bg4 = nc.dram_tensor("bg4", [1, BF], F16, kind="ExternalInput")  # bg tiled 4x

    out_t = nc.dram_tensor("out", [B, OUT], F32, kind="ExternalOutput")

    with tile.TileContext(nc) as tc:
        with (
            tc.tile_pool(name="const", bufs=1) as cpool,
            tc.tile_pool(name="sbuf", bufs=2) as pool,
            tc.tile_pool(name="psum", bufs=2, space="PSUM") as psum,
            tc.tile_pool(name="dram", bufs=1, space="DRAM") as dpool,
        ):
            # ---- constants / weights ----
            ident32 = cpool.tile([128, 128], F32)
            make_identity(nc, ident32[:])
            ident16 = cpool.tile([128, 128], F16)
            make_identity(nc, ident16[:])
            ones16 = cpool.tile([1, 128], F16)
            nc.vector.memset(ones16[:], 1.0)

            wpack_sb = cpool.tile([128, 849], F32)
            nc.sync.dma_start(out=wpack_sb[:], in_=wpack[:])
            bg4_sb = cpool.tile([1, BF], F16)
            nc.sync.dma_start(out=bg4_sb[:], in_=bg4[:])
            dlw_sb = cpool.tile([128, 2 * (total_slots // 128)], F32)
            nc.sync.dma_start(out=dlw_sb[:], in_=dlw[:])
            gidx_all = cpool.tile([128, total_slots // 16], I16)
            nc.sync.dma_start(out=gidx_all[:], in_=gidx[:])
            bn1all = cpool.tile([128, TPC, 2], F32)
            nc.sync.dma_start(out=bn1all[:],
                              in_=bn1p[:].rearrange("t p c -> p t c"))
            iota_sb = cpool.tile([128, 128], F16)
            nc.sync.dma_start(out=iota_sb[:], in_=iota16[:])
            wf1_sb = wpack_sb[:, 0:256]
            wf2a_sb = wpack_sb[:, 256:384]
            wf2b_sb = wpack_sb[:, 384:512]
            wg2_sb = wpack_sb[:, 512:640]
            wop2_sb = wpack_sb[0:SE_D, 640:768]
            w1_sb = wpack_sb[0:F, 768:800]
            w2_sb = wpack_sb[0:SE_D, 800:832]
            wo_sb = wpack_sb[:, 832:836]
            b1_sb = wpack_sb[0:SE_D, 836:837]
            b2_sb = wpack_sb[0:SE_D, 837:838]
            bop2_sb = wpack_sb[:, 838:839]
            bf1c_sb = wpack_sb[:, 839:841]
            bf2c_sb = wpack_sb[:, 841:842]
            g2_sb = wpack_sb[:, 842:844]
            be2_sb = wpack_sb[:, 844:846]
            g3_sb = wpack_sb[:, 846:847]
            be3_sb = wpack_sb[:, 847:848]
            boc_sb = wpack_sb[0:OUT, 848:849]

            # ---- phase G: SE gate (max-pool over nodes + tiny MLP) ----
            gates = None
            if "G" in phases:
                # sharded x-scan: each core reduces its N/8 slice, then
                # AllGather + local max of the [F, B] partials
                rows = N // NCORES
                q = rows // 4
                xs_sb = [cpool.tile([128, q // 128, BF], F16,
                                    tag=f"xs_sb{i}", name=f"xs_sb{i}")
                         for i in range(4)]
                reds = [cpool.tile([128, BF], F32, tag=f"reds{i}",
                                   name=f"reds{i}") for i in range(4)]
                for i in range(4):
                    nc.sync.dma_start(
                        out=xs_sb[i][:],
                        in_=xs[i * q:(i + 1) * q].rearrange(
                            "(p c) w -> p c w", p=128))
                    nc.vector.tensor_reduce(
                        out=reds[i][:],
                        in_=xs_sb[i][:].rearrange("p c w -> p w c"),
                        axis=AX.X, op=ALU.max,
                    )
                nc.vector.tensor_tensor(out=reds[0][:], in0=reds[0][:],
                                        in1=reds[1][:], op=ALU.max)
                nc.vector.tensor_tensor(out=reds[2][:], in0=reds[2][:],
                                        in1=reds[3][:], op=ALU.max)
                redpart = cpool.tile([128, BF], F32)
                nc.vector.tensor_tensor(out=redpart[:], in0=reds[0][:],
                                        in1=reds[2][:], op=ALU.max)
                pp = cpool.tile([F, B], F32)
                for b in range(B):
                    red_ps = psum.tile([F, 128], F32, space="PSUM",
                                       tag="ps_tr", bufs=3)
                    nc.tensor.transpose(
                        out=red_ps[:], in_=redpart[:, b * F:(b + 1) * F],
                        identity=ident32[:])
                    nc.vector.tensor_reduce(out=pp[:, b:b + 1], in_=red_ps[:],
                                            axis=AX.X, op=ALU.max)
                if skip_collective:
                    ppf = pp
                else:
                    r_in = dpool.tile([F, B], F32)
                    r_out = dpool.tile([NCORES, F, B], F32)
                    nc.gpsimd.dma_start(out=r_in[:], in_=pp[:])
                    nc.gpsimd.collective_compute(
                        "AllGather", ALU.bypass,
                        replica_groups=[list(range(NCORES))],
                        ins=[r_in.opt()], outs=[r_out.opt()],
                    )
                    ppg = cpool.tile([F, NCORES, B], F32)
                    nc.sync.dma_start(out=ppg[:],
                                      in_=r_out[:].rearrange("c p b -> p c b"))
                    ppf = cpool.tile([F, B], F32)
                    nc.vector.tensor_reduce(
                        out=ppf[:], in_=ppg[:].rearrange("p c b -> p b c"),
                        axis=AX.X, op=ALU.max)
                # gate MLP, all batches at once
                a1_ps = psum.tile([SE_D, B], F32, space="PSUM", tag="ps_b", bufs=1)
                nc.tensor.matmul(out=a1_ps[:], lhsT=w1_sb[:], rhs=ppf[:],
                                 start=True, stop=True)
                a1 = pool.tile([SE_D, B], F32, tag="a1")
                nc.scalar.activation(out=a1[:], in_=a1_ps[:], func=AF.Relu,
                                     bias=b1_sb[:])
                a2_ps = psum.tile([SE_D, B], F32, space="PSUM", tag="ps_b", bufs=1)
                nc.tensor.matmul(out=a2_ps[:], lhsT=w2_sb[:], rhs=a1[:],
                                 start=True, stop=True)
                a2 = pool.tile([SE_D, B], F32, tag="a2")
                nc.scalar.activation(out=a2[:], in_=a2_ps[:], func=AF.Relu,
                                     bias=b2_sb[:])
                g_ps = psum.tile([2 * F, B], F32, space="PSUM", tag="ps_b", bufs=1)
                nc.tensor.matmul(out=g_ps[:], lhsT=wop2_sb[:], rhs=a2[:],
                                 start=True, stop=True)
                gates = cpool.tile([2 * F, B], F32)
                nc.scalar.activation(out=gates[:], in_=g_ps[:],
                                     func=AF.Sigmoid, bias=bop2_sb[:])
                nc.vector.tensor_scalar_add(gates[:], gates[:], 1.0)

            # block-diagonal gate-scaled Wg pairs (fp16): for batch pair
            # (2j, 2j+1), rows 0:64 x cols 0:128 hold Wg*gate[2j] and rows
            # 64:128 x cols 128:256 hold Wg*gate[2j+1]. One h2 matmul then
            # computes both batches (contraction over the stacked f axis).
            wgebz = []
            for j in range(4):
                wz = cpool.tile([2 * F, 2 * H], F16, tag=f"wgebz_{j}",
                                name=f"wgebz_{j}")
                nc.vector.memset(wz[:], 0.0)
                if gates is not None:
                    nc.vector.tensor_scalar(
                        out=wz[0:F, 0:H], in0=wg2_sb[0:F, :],
                        scalar1=gates[0:F, 2 * j:2 * j + 1],
                        scalar2=None, op0=ALU.mult)
                    nc.vector.tensor_scalar(
                        out=wz[F:2 * F, H:2 * H], in0=wg2_sb[F:2 * F, :],
                        scalar1=gates[F:2 * F, 2 * j + 1:2 * j + 2],
                        scalar2=None, op0=ALU.mult)
                else:
                    nc.vector.tensor_copy(out=wz[0:F, 0:H],
                                          in_=wg2_sb[0:F, :])
                    nc.vector.tensor_copy(out=wz[F:2 * F, H:2 * H],
                                          in_=wg2_sb[F:2 * F, :])
                wgebz.append(wz)

            # ---- phase B: SpMM + Wg + BN1 + pool partial ----
            pooled = cpool.tile([H, B], F32)
            poolacc = [cpool.tile([128, BF], F16, tag=f"poolacc_{g}",
                                  name=f"poolacc_{g}")
                       for g in range(2)]

            for t in range(TPC) if "B" in phases else []:
                cpt_t = cpts[t]
                slots_t = slots_i[t]
                off_t = int(offs_i[t])

                # build the one-hot S blocks on-chip: row e of chunk k gets
                # w at column dloc ((iota == dloc) * w), zeros elsewhere
                s_sb = pool.tile([128, slots_t], F16, tag="s_sb", bufs=5,
                                 name="s_sb")
                cbase = off_t // 128
                for k in range(cpt_t):
                    c = cbase + k
                    nc.vector.tensor_scalar(
                        out=s_sb[:, k * 128:(k + 1) * 128], in0=iota_sb[:],
                        scalar1=dlw_sb[:, 2 * c:2 * c + 1],
                        scalar2=dlw_sb[:, 2 * c + 1:2 * c + 2],
                        op0=ALU.is_equal, op1=ALU.mult)

                msg = pool.tile([128, cpt_t, BF], F16, tag="msg", bufs=5,
                                name="msg")
                for s0 in range(0, slots_t, MAX_GATHER):
                    n_i = min(MAX_GATHER, slots_t - s0)
                    nc.gpsimd.dma_gather(
                        out_ap=msg[:, s0 // 128:(s0 + n_i) // 128, :],
                        in_ap=xt[:],
                        idxs_ap=gidx_all[:, (off_t + s0) // 16:(off_t + s0 + n_i) // 16],
                        num_idxs=n_i, num_idxs_reg=n_i, elem_size=BF,
                    )
                agg_ps = psum.tile([128, BF], F32, space="PSUM", tag="ps_agg",
                                   bufs=2)
                for k in range(cpt_t):
                    nc.tensor.matmul(
                        out=agg_ps[:],
                        lhsT=s_sb[:, k * 128:(k + 1) * 128],
                        rhs=msg[:, k, :],
                        start=(k == 0), stop=(k == cpt_t - 1),
                    )
                agg32 = pool.tile([128, BF], F32, tag="agg32", bufs=4,
                                  name="agg32")
                nc.scalar.activation(out=agg32[:], in_=agg_ps[:], func=AF.Copy)
                # batch-pair transposes; aggT[:, j, :] holds batches 2j
                # (partitions 0:64) and 2j+1 (64:128), fp16-cast on evacuation
                aggT = pool.tile([128, 4, 128], F16, tag="aggT", bufs=8,
                                 name="aggT")
                for j in range(4):
                    tr_ps = psum.tile([128, 128], F32, space="PSUM",
                                      tag="ps_tr", bufs=3)
                    nc.tensor.transpose(
                        out=tr_ps[:], in_=agg32[:, j * 128:(j + 1) * 128],
                        identity=ident32[:])
                    nc.scalar.activation(out=aggT[:, j, :], in_=tr_ps[:],
                                         func=AF.Copy)
                # h2 = relu(agg_gated @ Wg + bg), 4 batches per PSUM bank via
                # block-diagonal pair weights
                sums = pool.tile([128, 2], F32, tag="sums", bufs=4, name="sums")
                sqs = pool.tile([128, 2], F32, tag="sqs", bufs=4, name="sqs")
                h2g = []
                for g in range(2):
                    h2_ps = psum.tile([128, BF], F32, space="PSUM",
                                      tag="ps_h2", bufs=2)
                    nc.tensor.matmul(out=h2_ps[:], lhsT=ones16[:],
                                     rhs=bg4_sb[:], start=True, stop=False)
                    for jj in range(2):
                        j = g * 2 + jj
                        nc.tensor.matmul(
                            out=h2_ps[:, jj * 2 * H:(jj + 1) * 2 * H],
                            lhsT=aggT[:, j, :],
                            rhs=wgebz[j][:],
                            start=False, stop=(jj == 1))
                    h2 = pool.tile([128, BF], F16, tag=f"h2_{g}", bufs=4,
                                   name=f"h2_{g}")
                    nc.scalar.activation(out=h2[:], in_=h2_ps[:], func=AF.Relu,
                                         accum_out=sums[:, g:g + 1])
                    sqscr = pool.tile([128, BF], F16, tag="sqscr", bufs=4,
                                      name="sqscr")
                    nc.vector.tensor_tensor(out=sqscr[:], in0=h2[:],
                                            in1=h2[:], op=ALU.mult)
                    nc.vector.tensor_reduce(out=sqs[:, g:g + 1],
                                            in_=sqscr[:], axis=AX.X,
                                            op=ALU.add)
                    h2g.append(h2)
                # BN1 per-node affine coefficients (DVE column math)
                rsumt = pool.tile([128, 1], F32, tag="rsumt", bufs=4, name="rsumt")
                nc.vector.tensor_reduce(out=rsumt[:], in_=sums[:], axis=AX.X,
                                        op=ALU.add)
                sqsumt = pool.tile([128, 1], F32, tag="sqsumt", bufs=4, name="sqsumt")
                nc.vector.tensor_reduce(out=sqsumt[:], in_=sqs[:], axis=AX.X,
                                        op=ALU.add)
                mean = pool.tile([128, 1], F32, tag="mean", bufs=4, name="mean")
                nc.vector.tensor_scalar_mul(mean[:], rsumt[:], 1.0 / (B * H))
                msqe = pool.tile([128, 1], F32, tag="msqe", bufs=4, name="msqe")
                nc.vector.tensor_scalar_mul(msqe[:], sqsumt[:], 1.0 / (B * H))
                nc.vector.tensor_scalar_add(msqe[:], msqe[:], BN_EPS)
                var = pool.tile([128, 1], F32, tag="var", bufs=4, name="var")
                nc.vector.tensor_tensor(out=var[:], in0=mean[:], in1=mean[:],
                                        op=ALU.mult)
                nc.vector.tensor_tensor(out=var[:], in0=msqe[:], in1=var[:],
                                        op=ALU.subtract)
                inv = pool.tile([128, 1], F32, tag="inv", bufs=4, name="inv")
                nc.vector.reciprocal(out=inv[:], in_=var[:])
                rstd = pool.tile([128, 1], F32, tag="rstd", bufs=4, name="rstd")
                nc.scalar.sqrt(out=rstd[:], in_=inv[:])
                aco = pool.tile([128, 1], F32, tag="aco", bufs=4, name="aco")
                nc.vector.tensor_tensor(out=aco[:], in0=rstd[:],
                                        in1=bn1all[:, t, 0:1], op=ALU.mult)
                bco = pool.tile([128, 1], F32, tag="bco", bufs=4, name="bco")
                nc.vector.tensor_tensor(out=bco[:], in0=mean[:], in1=aco[:],
                                        op=ALU.mult)
                nc.vector.tensor_tensor(out=bco[:], in0=bn1all[:, t, 1:2],
                                        in1=bco[:], op=ALU.subtract)
                # apply BN1 (ACT: out = aco*h2 + bco) and fold into pool max
                for g in range(2):
                    if t == 0:
                        nc.vector.tensor_scalar(
                            out=poolacc[g][:], in0=h2g[g][:],
                            scalar1=aco[:, 0:1], scalar2=bco[:, 0:1],
                            op0=ALU.mult, op1=ALU.add)
                    else:
                        h2n = pool.tile([128, BF], F16, tag="h2n", bufs=4,
                                        name="h2n")
                        nc.vector.tensor_scalar(
                            out=h2n[:], in0=h2g[g][:],
                            scalar1=aco[:, 0:1], scalar2=bco[:, 0:1],
                            op0=ALU.mult, op1=ALU.add)
                        nc.vector.tensor_tensor(out=poolacc[g][:],
                                                in0=poolacc[g][:], in1=h2n[:],
                                                op=ALU.max)

            # fold pooled partials: per batch, transpose + reduce over nodes
            pacc32 = [cpool.tile([128, BF], F32, tag=f"pacc32_{g}",
                                 name=f"pacc32_{g}") for g in range(2)]
            for g in range(2):
                nc.scalar.activation(out=pacc32[g][:], in_=poolacc[g][:],
                                     func=AF.Copy)
            for b in range(B):
                g, j = b // 4, b % 4
                hT_ps = psum.tile([128, 128], F32, space="PSUM", tag="ps_tr", bufs=3)
                nc.tensor.transpose(
                    out=hT_ps[:], in_=pacc32[g][:, j * H:(j + 1) * H],
                    identity=ident32[:])
                nc.vector.tensor_reduce(out=pooled[:, b:b + 1], in_=hT_ps[:],
                                        axis=AX.X, op=ALU.max)

            # ---- phase C: AllGather + local max + replicated head ----
            if skip_collective:
                pooledf = pooled
            else:
                p_in = dpool.tile([H, B], F32)
                p_out = dpool.tile([NCORES, H, B], F32)
                nc.gpsimd.dma_start(out=p_in[:], in_=pooled[:])
                nc.gpsimd.collective_compute(
                    "AllGather", ALU.bypass,
                    replica_groups=[list(range(NCORES))],
                    ins=[p_in.opt()], outs=[p_out.opt()],
                )
                pg = cpool.tile([H, NCORES, B], F32)
                nc.sync.dma_start(out=pg[:],
                                  in_=p_out[:].rearrange("c p b -> p c b"))
                pooledf = cpool.tile([H, B], F32)
                nc.vector.tensor_reduce(
                    out=pooledf[:], in_=pg[:].rearrange("p c b -> p b c"),
                    axis=AX.X, op=ALU.max)

            def bn_free8(z, nrows, gamma_col, beta_col, tag):
                """BN over the 8 free-dim entries of z [nrows, 8] -> new tile."""
                rs = pool.tile([nrows, 1], F32, tag=f"{tag}_rs")
                nc.vector.tensor_reduce(out=rs[:], in_=z[:], axis=AX.X,
                                        op=ALU.add)
                nc.vector.tensor_scalar_mul(rs[:], rs[:], 1.0 / B)
                sqt = pool.tile([nrows, B], F32, tag=f"{tag}_sqt")
                sq = pool.tile([nrows, 1], F32, tag=f"{tag}_sq")
                nc.scalar.activation(out=sqt[:], in_=z[:], func=AF.Square,
                                     accum_out=sq[:])
                nc.vector.tensor_scalar_mul(sq[:], sq[:], 1.0 / B)
                v = pool.tile([nrows, 1], F32, tag=f"{tag}_v")
                nc.vector.tensor_tensor(out=v[:], in0=rs[:], in1=rs[:],
                                        op=ALU.mult)
                nc.vector.tensor_tensor(out=v[:], in0=sq[:], in1=v[:],
                                        op=ALU.subtract)
                nc.vector.tensor_scalar_add(v[:], v[:], BN_EPS)
                iv = pool.tile([nrows, 1], F32, tag=f"{tag}_iv")
                nc.vector.reciprocal(out=iv[:], in_=v[:])
                rst = pool.tile([nrows, 1], F32, tag=f"{tag}_rst")
                nc.scalar.sqrt(out=rst[:], in_=iv[:])
                ac = pool.tile([nrows, 1], F32, tag=f"{tag}_ac")
                nc.vector.tensor_tensor(out=ac[:], in0=rst[:], in1=gamma_col,
                                        op=ALU.mult)
                bc = pool.tile([nrows, 1], F32, tag=f"{tag}_bc")
                nc.vector.tensor_tensor(out=bc[:], in0=rs[:], in1=ac[:],
                                        op=ALU.mult)
                nc.vector.tensor_tensor(out=bc[:], in0=beta_col, in1=bc[:],
                                        op=ALU.subtract)
                zn = pool.tile([nrows, B], F32, tag=f"{tag}_zn")
                nc.vector.tensor_scalar(out=zn[:], in0=z[:],
                                        scalar1=ac[:, 0:1], scalar2=bc[:, 0:1],
                                        op0=ALU.mult, op1=ALU.add)
                return zn

            # head in feature-major layout: features on partitions, batch on
            # free. Biases ride ACT's per-partition bias port; no transposes.
            z1n = []
            for j in range(2):
                z1_ps = psum.tile([H, B], F32, space="PSUM", tag="ps_b", bufs=1)
                nc.tensor.matmul(out=z1_ps[:],
                                 lhsT=wf1_sb[:, j * 128:(j + 1) * 128],
                                 rhs=pooledf[:], start=True, stop=True)
                z1T = pool.tile([H, B], F32, tag=f"z1T_{j}")
                nc.scalar.activation(out=z1T[:], in_=z1_ps[:], func=AF.Relu,
                                     bias=bf1c_sb[:, j:j + 1])
                z1n.append(bn_free8(z1T, 128, g2_sb[:, j:j + 1],
                                    be2_sb[:, j:j + 1], f"bn2_{j}"))
            z2_ps = psum.tile([FC2, B], F32, space="PSUM", tag="ps_b", bufs=1)
            nc.tensor.matmul(out=z2_ps[:], lhsT=wf2a_sb[:], rhs=z1n[0][:],
                             start=True, stop=False)
            nc.tensor.matmul(out=z2_ps[:], lhsT=wf2b_sb[:], rhs=z1n[1][:],
                             start=False, stop=True)
            z2T = pool.tile([FC2, B], F32, tag="z2T")
            nc.scalar.activation(out=z2T[:], in_=z2_ps[:], func=AF.Relu,
                                 bias=bf2c_sb[:])
            z2n = bn_free8(z2T, FC2, g3_sb[:, 0:1], be3_sb[:, 0:1], "bn3")
            # logits^T [OUT, B], add bias, transpose to [B, OUT], softmax
            lg_ps = psum.tile([OUT, B], F32, space="PSUM", tag="ps_b", bufs=1)
            nc.tensor.matmul(out=lg_ps[:], lhsT=wo_sb[:], rhs=z2n[:],
                             start=True, stop=True)
            lgT = pool.tile([OUT, B], F32, tag="lgT")
            nc.vector.tensor_scalar(out=lgT[:], in0=lg_ps[:],
                                    scalar1=boc_sb[:, 0:1], scalar2=None,
                                    op0=ALU.add)
            lgt_ps = psum.tile([B, OUT], F32, space="PSUM", tag="ps_b", bufs=1)
            nc.tensor.transpose(out=lgt_ps[:], in_=lgT[:],
                                identity=ident32[0:OUT, 0:OUT])
            lg = pool.tile([B, OUT], F32, tag="lg")
            nc.vector.tensor_copy(out=lg[:], in_=lgt_ps[:])
            mx = pool.tile([B, 1], F32, tag="mx")
            nc.vector.tensor_reduce(out=mx[:], in_=lg[:], axis=AX.X, op=ALU.max)
            ex = pool.tile([B, OUT], F32, tag="ex")
            nc.vector.tensor_scalar(out=ex[:], in0=lg[:], scalar1=mx[:, 0:1],
                                    scalar2=None, op0=ALU.subtract)
            nc.scalar.activation(out=ex[:], in_=ex[:], func=AF.Exp)
            ssum = pool.tile([B, 1], F32, tag="ssum")
            nc.vector.tensor_reduce(out=ssum[:], in_=ex[:], axis=AX.X,
                                    op=ALU.add)
            sinv = pool.tile([B, 1], F32, tag="sinv")
            nc.vector.reciprocal(out=sinv[:], in_=ssum[:])
            sm = pool.tile([B, OUT], F32, tag="sm")
            nc.vector.tensor_scalar(out=sm[:], in0=ex[:], scalar1=sinv[:, 0:1],
                                    scalar2=None, op0=ALU.mult)
            nc.sync.dma_start(out=out_t[:], in_=sm[:])
    nc.compile()
    return nc


def preprocess(x, src, dst, edge_w):
    """Host marshalling: node-major x table + sorted/padded edge tiles."""
    order = np.argsort(dst, kind="stable")
    ss = src[order].astype(np.int64)
    ds = dst[order].astype(np.int64)
    ws = edge_w[order].astype(np.float32)
    tile_id = ds // 128
    dloc = ds % 128
    counts = np.bincount(tile_id, minlength=NTILE)
    cpt = int(np.ceil(counts.max() / 128))
    slots = cpt * 128

    gidx_all = np.zeros((NTILE, slots), np.int16)
    sval = np.zeros((NTILE, slots), np.float32)
    sloc = np.zeros((NTILE, slots), np.int64)
    offs = np.concatenate([[0], np.cumsum(counts)])
    for t in range(NTILE):
        cnt = counts[t]
        seg = slice(offs[t], offs[t + 1])
        # order each tile's edges by ascending src so gather descriptors
        # read ascending HBM addresses (DRAM row-buffer locality); the
        # segment-sum is order-invariant since S follows the slot order
        o = np.argsort(ss[seg], kind="stable")
        gidx_all[t, :cnt] = ss[seg][o]
        sval[t, :cnt] = ws[seg][o]
        sloc[t, :cnt] = dloc[seg][o]

    # per-chunk (dloc, w) columns; padded slots get dloc=200 (unmatchable)
    dlocw = np.zeros((NTILE, 128, 2 * cpt), np.float32)
    for t in range(NTILE):
        dl = sloc[t].reshape(cpt, 128).T.astype(np.float32)
        wv = sval[t].reshape(cpt, 128).T
        dl[wv == 0.0] = 200.0
        dlocw[t, :, 0::2] = dl
        dlocw[t, :, 1::2] = wv
    dlocw = dlocw.astype(np.float32)

    # wrapped int16 index tables: [16, slots//16] replicated to 128 partitions
    gidx_w = np.zeros((NTILE, 128, slots // 16), np.int16)
    for t in range(NTILE):
        base = gidx_all[t].reshape(slots // 16, 16).T
        gidx_w[t] = np.tile(base, (8, 1))

    # per-core tile order (descending edge count) and per-slot chunk counts
    order_pc = np.zeros((NCORES, TPC), np.int64)
    for c in range(NCORES):
        tl = np.arange(c * TPC, (c + 1) * TPC)
        order_pc[c] = tl[np.argsort(-counts[tl], kind="stable")]
    cpts = tuple(
        int(np.ceil(max(counts[order_pc[c][i]] for c in range(NCORES)) / 128))
        for i in range(TPC)
    )
    xt = np.ascontiguousarray(
        np.asarray(x, np.float32).transpose(1, 0, 2).reshape(N, BF)
    ).astype(f16)
    return xt, gidx_w, dlocw, cpts, order_pc


def make_in_maps(inputs, xt, gidx_w, dlocw, cpts, order_pc):
    g1 = np.asarray(inputs["g1"], np.float32).reshape(NTILE, 128)
    beta1 = np.asarray(inputs["beta1"], np.float32).reshape(NTILE, 128)
    bn1 = np.stack([g1, beta1], axis=-1)  # [NTILE, 128, 2]

    f32 = lambda a: np.ascontiguousarray(np.asarray(a, np.float32))
    wp = np.zeros((128, 849), np.float32)
    wp[:, 0:256] = f32(inputs["Wf1"])
    wf2 = f32(inputs["Wf2"]).reshape(2, H, FC2)
    wp[:, 256:384] = wf2[0]
    wp[:, 384:512] = wf2[1]
    wp[:, 512:640] = np.tile(f32(inputs["Wg"]), (2, 1))
    wp[0:SE_D, 640:768] = np.concatenate([f32(inputs["Wop"])] * 2, axis=1)
    wp[0:F, 768:800] = f32(inputs["W1"])
    wp[0:SE_D, 800:832] = f32(inputs["W2"])
    wp[:, 832:836] = f32(inputs["Wo"])
    wp[0:SE_D, 836] = f32(inputs["b1"])
    wp[0:SE_D, 837] = f32(inputs["b2"])
    wp[:, 838] = np.tile(f32(inputs["bop"]), 2)
    wp[:, 839:841] = f32(inputs["bf1"]).reshape(2, H).T
    wp[:, 841] = f32(inputs["bf2"])
    wp[:, 842:844] = f32(inputs["g2"]).reshape(2, H).T
    wp[:, 844:846] = f32(inputs["beta2"]).reshape(2, H).T
    wp[:, 846] = f32(inputs["g3"])
    wp[:, 847] = f32(inputs["beta3"])
    wp[0:OUT, 848] = f32(inputs["bo"])
    shared = {
        "xt": xt,
        "iota16": np.tile(np.arange(128, dtype=np.float32), (128, 1)).astype(f16),
        "wpack": wp,
        "bg4": np.tile(f32(inputs["bg"]).reshape(1, H), (1, 4)).astype(f16),
    }
    in_maps = []
    for c in range(NCORES):
        order = order_pc[c]
        m = dict(shared)
        m["xs"] = np.ascontiguousarray(xt[c * (N // NCORES):(c + 1) * (N // NCORES)])
        m["gidx"] = np.ascontiguousarray(np.concatenate(
            [gidx_w[gt][:, :cpts[i] * 8] for i, gt in enumerate(order)], axis=1))
        m["dlw"] = np.ascontiguousarray(np.concatenate(
            [dlocw[gt][:, :2 * cpts[i]] for i, gt in enumerate(order)], axis=1))
        m["bn1p"] = np.ascontiguousarray(bn1[order])
        in_maps.append(m)
    return in_maps


_CACHE = {}
LAST_RESULT = None  # BassKernelResults of the most recent kernel() call


def kernel(**inputs):
    global LAST_RESULT
    xt, gidx_w, dlocw, cpts, order_pc = preprocess(
        np.asarray(inputs["x"]), np.asarray(inputs["src"]),
        np.asarray(inputs["dst"]), np.asarray(inputs["edge_w"]))
    if cpts not in _CACHE:
        _CACHE[cpts] = build_kernel(cpts)
    nc = _CACHE[cpts]
    in_maps = make_in_maps(inputs, xt, gidx_w, dlocw, cpts, order_pc)
    trace = os.environ.get("BASS_KERNEL_TRACE", "0") == "1"
    res = run_bass_kernel_spmd(nc, in_maps, list(range(NCORES)), trace=trace)
    LAST_RESULT = res
    return np.asarray(res.results[0]["out"], np.float32)
